# revision 28
# baseline (speedup 1.0000x reference)
"""Causal self-attention (B=2, T=4096, C=768, H=12) on 8 TRN2 NeuronCores.

Sharding: batch x head-group. Core c handles batch b=c//4 and heads
h0..h0+2 where h0 = 3*(c%4). Each core computes the qkv projection for
its 3 heads, full causal attention, and a partial output projection; the
host sums the 4 partials per batch and adds the (augmented) projection
bias.

Numerics / structure:
- All matmul operands are bf16 (f32 PSUM accumulation).
- k-bias is dropped entirely: softmax over k is invariant to the
  per-query constant q . b_k. v-bias is folded into the host-side output
  bias (softmax weights sum to 1, so + b_v @ W_proj).
- q/k live transposed ([d, T]) feeding the scores matmul; v is computed
  directly in natural layout ([T, d]) with an interleaved ones column
  per head providing the softmax denominator through the att@v matmul.
- scores^T tiles [128 k, 512 q] -> exp -> eb (bf16). exp runs on the Act
  engine for ~3/4 of tiles and as a Schraudolph int16-bitcast
  approximation (tensor_scalar mult+add, then f32->int16 convert copy)
  on DVE for the rest, keeping both engines busy.
- att@v is "flipped": out [128 q, 65] accumulated over k-tiles (65
  cycles per matmul instead of 512), using eb chunks as the stationary
  operand. PSUM accumulation uses pre-zeroed banks + start=False
  matmuls: a start=True wipes the whole bank's in-flight accumulation,
  so interleaved per-chunk groups in one bank need memory-accumulate.
  Diagonal tiles skip fully-masked chunks/columns.
- Normalization uses the denominator column as a per-partition scalar
  (reciprocal + tensor_scalar_mul), then PE transposes the normalized
  [q, d] chunks to d-major for the output projection.
- Phase 3 is software-pipelined: scores/exp for tile ki are emitted one
  step ahead of the att@v consuming ki-1, so att@v sem-waits (4-deep PE
  wait station) never block the next tile's score matmuls.
"""

import sys

for _p in ("/opt/trn_rl_repo",):
    if _p not in sys.path:
        sys.path.insert(0, _p)

import math
import os
from contextlib import ExitStack

import numpy as np
import ml_dtypes

import concourse.bass as bass
import concourse.mybir as mybir
import concourse.tile as tile
from concourse import bacc
from concourse.bass_utils import run_bass_kernel_spmd
from concourse.masks import make_identity

f32 = mybir.dt.float32
bf16 = mybir.dt.bfloat16
i16 = mybir.dt.int16
AF = mybir.ActivationFunctionType
Alu = mybir.AluOpType

C = 768
D = 64
HPC = 3  # heads per core
N_CORES = 8
CK = C // 128  # contraction chunks

SCALE = 1.0 / math.sqrt(D)  # 0.125
# Schraudolph: exp(s*SCALE) ~ bitcast_bf16(int16(s*A_SCH + B_SCH))
A_SCH = SCALE * 128.0 / math.log(2.0)
B_SCH = 128.0 * (127.0 - 0.043)

# exp path pattern over score tiles: True -> Act exp, False -> DVE
# Schraudolph (2 DVE ops).
EXP_PAT = [True, True, False]


def build_nc(T):
    NT = T // 512  # q tiles
    KT = T // 128  # k tiles
    T2 = T // 2

    nc = bacc.Bacc("TRN2", target_bir_lowering=False, debug=False,
                   num_devices=N_CORES)
    xt_d = nc.dram_tensor("xt", [C, T], bf16, kind="ExternalInput").ap()
    wq_d = nc.dram_tensor("wq", [C, 576], bf16, kind="ExternalInput").ap()
    bq_d = nc.dram_tensor("bq", [128, 2], f32, kind="ExternalInput").ap()
    wp01_d = nc.dram_tensor("wp01", [128, C], bf16, kind="ExternalInput").ap()
    wp2_d = nc.dram_tensor("wp2", [64, C], bf16, kind="ExternalInput").ap()
    y_d = nc.dram_tensor("y", [T, C], f32, kind="ExternalOutput").ap()
    dbg = os.environ.get("KDBG") == "1"
    dbg_out = {}
    if dbg:
        for nm, shp in [("d_qAB", [128, T]), ("d_kAB", [128, T]),
                        ("d_qC", [64, T]), ("d_kC2", [64, T]),
                        ("d_vaug", [128, KT * 195]),
                        ("d_ao01", [128, T]), ("d_ao2", [64, T]),
                        ("d_eb", [128, 1024]), ("d_att", [128, 512])]:
            dbg_out[nm] = nc.dram_tensor(nm, shp, f32, kind="ExternalOutput").ap()

    with tile.TileContext(nc) as tc, ExitStack() as ctx:
        sb = ctx.enter_context(tc.tile_pool(name="sb", bufs=1))

        # persistent tensors
        bq_sb = sb.tile([128, 2], f32, tag="bq")
        qT_AB = sb.tile([128, T], bf16, tag="qAB")
        kT_AB = sb.tile([128, T], bf16, tag="kAB")
        qT_C = sb.tile([64, T], bf16, tag="qC")
        kC2 = sb.tile([64, T], bf16, tag="kC2")
        v_aug = sb.tile([128, KT * 195], bf16, tag="vaug")
        aoT01 = sb.tile([128, T], bf16, tag="aoT01")
        aoT2 = sb.tile([64, T], bf16, tag="aoT2")
        ident = sb.tile([128, 128], bf16, tag="ident")
        cmask = sb.tile([128, 128], bf16, tag="cmask")

        nc.sync.dma_start(bq_sb[:], bq_d)
        make_identity(nc, ident[:])
        # causal triangle for the in-diagonal 128-col strip: keep col >= part
        nc.gpsimd.memset(cmask[:], 1.0)
        nc.gpsimd.affine_select(
            cmask[:], cmask[:], pattern=[[1, 128]],
            compare_op=Alu.is_ge, fill=0.0, base=0, channel_multiplier=-1)
        # ones columns of v_aug (denominator rows through att@v)
        ones_cols = v_aug[:].rearrange("p (k h c) -> p k h c", h=3, c=65)[:, :, :, 64:65]
        nc.gpsimd.memset(ones_cols, 1.0)

        wpp = ctx.enter_context(tc.tile_pool(name="wpp", bufs=1))
        wp01 = wpp.tile([128, C], bf16, tag="wp01")
        wp2 = wpp.tile([64, C], bf16, tag="wp2")

        es_p1 = ExitStack()
        xtp = es_p1.enter_context(tc.tile_pool(name="xtp", bufs=1))
        wqp = es_p1.enter_context(tc.tile_pool(name="wqp", bufs=1))
        k2p = es_p1.enter_context(tc.tile_pool(name="k2p", bufs=1))
        xt_sb = [xtp.tile([128, T], bf16, tag=f"xt{c}", name=f"xt{c}")
                 for c in range(CK)]
        wq_sb = [wqp.tile([128, 576], bf16, tag=f"wq{c}", name=f"wq{c}")
                 for c in range(CK)]
        k2s = k2p.tile([128, T], bf16, tag="k2s")
        for c in range(CK):
            nc.sync.dma_start(wq_sb[c][:], wq_d[c * 128:(c + 1) * 128, :])
        for c in range(CK):
            nc.sync.dma_start(xt_sb[c][:, 0:512],
                              xt_d[c * 128:(c + 1) * 128, 0:512])
        for h0_, h1_ in ((512, T2), (T2, T)):
            if h1_ <= h0_:
                continue
            for c in range(CK):
                nc.sync.dma_start(xt_sb[c][:, h0_:h1_],
                                  xt_d[c * 128:(c + 1) * 128, h0_:h1_])
        nc.sync.dma_start(wp01[:], wp01_d)
        nc.sync.dma_start(wp2[:], wp2_d)

        # ---------------- phase 1: qkv projection --------------------------
        with tc.tile_pool(name="qkp", bufs=2, space="PSUM") as qkp, \
             tc.tile_pool(name="vps", bufs=2, space="PSUM") as vps:
            for j in range(NT):
                jsl = bass.ts(j, 512)
                q01 = qkp.tile([128, 512], f32, tag="q01", name="q01")
                k01 = qkp.tile([128, 512], f32, tag="k01", name="k01")
                qk2 = qkp.tile([128, 512], f32, tag="qk2", name="qk2")
                for c in range(CK):
                    st, sp = c == 0, c == CK - 1
                    nc.tensor.matmul(q01[:], wq_sb[c][:, 0:128],
                                     xt_sb[c][:, jsl], start=st, stop=sp)
                    nc.tensor.matmul(k01[:], wq_sb[c][:, 128:256],
                                     xt_sb[c][:, jsl], start=st, stop=sp)
                    nc.tensor.matmul(qk2[:], wq_sb[c][:, 256:384],
                                     xt_sb[c][:, jsl], start=st, stop=sp)
                nc.vector.tensor_scalar_add(qT_AB[:, jsl], q01[:],
                                            bq_sb[:, 0:1])
                nc.vector.tensor_copy(kT_AB[:, jsl], k01[:])
                nc.vector.tensor_scalar_add(qT_C[0:64, jsl], qk2[0:64, :],
                                            bq_sb[0:64, 1:2])
                nc.vector.tensor_copy(k2s[64:128, jsl], qk2[64:128, :])
                for mi in range(4):
                    m = 4 * j + mi
                    msl = bass.ts(m, 128)
                    vp = vps.tile([128, 192], f32, tag="vp", name="vp")
                    for c in range(CK):
                        nc.tensor.matmul(vp[:], xt_sb[c][:, msl],
                                         wq_sb[c][:, 384:576],
                                         start=(c == 0), stop=(c == CK - 1))
                    vdst = v_aug[:, m * 195:(m + 1) * 195].rearrange(
                        "p (h c) -> p h c", c=65)[:, :, 0:64]
                    nc.vector.tensor_copy(
                        vdst, vp[:].rearrange("p (h c) -> p h c", c=64))
        # k2: partition shift 64:128 -> 0:64 via sbuf->sbuf DMA
        nc.sync.dma_start(kC2[0:64, :], k2s[64:128, :])
        es_p1.close()
        if dbg:
            for nm, t_ in [("d_qAB", qT_AB), ("d_kAB", kT_AB),
                           ("d_vaug", v_aug)]:
                tmp = sb.tile(list(t_.shape), f32, tag=f"t{nm}", name=f"t{nm}")
                nc.vector.tensor_copy(tmp[:], t_[:])
                nc.sync.dma_start(dbg_out[nm], tmp[:])
            for nm, t_ in [("d_qC", qT_C), ("d_kC2", kC2)]:
                tmp = sb.tile([64, T], f32, tag=f"t{nm}", name=f"t{nm}")
                nc.vector.tensor_copy(tmp[0:64, :], t_[0:64, :])
                nc.sync.dma_start(dbg_out[nm], tmp[0:64, :])

        # ---------------- phase 3: attention + projection ------------------
        with tc.tile_pool(name="scp", bufs=2, space="PSUM") as scp, \
             tc.tile_pool(name="attp", bufs=2, space="PSUM") as attp, \
             tc.tile_pool(name="trp", bufs=1, space="PSUM") as trp, \
             tc.tile_pool(name="ebp", bufs=7) as ebp, \
             tc.tile_pool(name="ebfp", bufs=3) as ebfp, \
             tc.tile_pool(name="anp", bufs=2) as anp, \
             tc.tile_pool(name="rcpp", bufs=6) as rcpp, \
             tc.tile_pool(name="ysp", bufs=4) as ysp:

            # 2 banks of manually sub-allocated scratch: transposes (bf16
            # view of f32 cols 0:256, h01/h2 sequential) + projection psum
            # thirds (pyA f32 cols 256:512, pyB 512:768). Tile dep-tracking
            # is range-precise, so disjoint sub-ranges don't serialize.
            trx = trp.tile([128, 1024], f32, tag="trx", name="trx")
            trx_bf = trx[:, 0:256].bitcast(bf16)  # [128, 512] bf16
            exp_ctr = [0]

            def emit_exp(specs):
                """specs: list of (sc_ap, eb_ap, ebf_cols) with matching
                shapes; one score tile, routed to Act or DVE. Returns True
                if the Act path was used."""
                use_act = EXP_PAT[exp_ctr[0] % len(EXP_PAT)]
                exp_ctr[0] += 1
                if use_act:
                    for sc_ap, eb_ap, _ in specs:
                        nc.scalar.activation(eb_ap, sc_ap, AF.Exp,
                                             scale=SCALE)
                else:
                    ebf = ebfp.tile([128, 1024], f32, tag="ebf", name="ebf")
                    for sc_ap, eb_ap, cols in specs:
                        ebf_ap = ebf[:, cols]
                        if len(sc_ap.shape) == 3:
                            ebf_ap = ebf_ap.rearrange(
                                "p (h c) -> p h c", h=sc_ap.shape[1])
                        nc.vector.tensor_scalar(ebf_ap, sc_ap,
                                                A_SCH, B_SCH,
                                                Alu.mult, Alu.add)
                        nc.gpsimd.tensor_copy(eb_ap.bitcast(i16), ebf_ap)
                return use_act

            proj_pending = []
            py_ctr = [0]

            def emit_proj():
                if not proj_pending:
                    return
                m = proj_pending.pop(0)
                msl = bass.ts(m, 128)
                y_sb = ysp.tile([128, C], f32, tag="ysb", name="ysb")
                for third in range(3):
                    off = 256 + 256 * (py_ctr[0] % 2)
                    py_ctr[0] += 1
                    py = trx[:, off:off + 256]
                    csl = slice(third * 256, (third + 1) * 256)
                    nc.tensor.matmul(py, aoT01[:, msl], wp01[:, csl],
                                     start=True, stop=False,
                                     skip_group_check=True)
                    nc.tensor.matmul(py, aoT2[0:64, msl], wp2[0:64, csl],
                                     start=False, stop=True,
                                     skip_group_check=True)
                    nc.vector.tensor_copy(y_sb[:, csl], py)
                nc.sync.dma_start(y_d[m * 128:(m + 1) * 128, :], y_sb[:])

            # ---- flat software-pipelined tile stream across all j,
            # heads 0,1 (per k-tile) and head 2 (per k-tile pair).
            # Emission order per step: consume(i) -> posts(i) ->
            # produce(i+2), keeping 2 score tiles in flight so att@v never
            # waits on exp and PSUM slot reuse (WAR) never blocks.
            stream = []
            for j in range(NT):
                nk = 4 * j + 4
                for ki in range(nk):
                    stream.append(("01", j, ki))
                for kp in range(nk // 2):
                    stream.append(("2", j, kp))
            NTILES = len(stream)

            st = {}  # per-j live tiles: att01, att2, eb tiles

            def produce(i):
                kind, j, k = stream[i]
                nk = 4 * j + 4
                if kind == "01":
                    ki = k
                    if ki == 0:
                        att01 = [attp.tile([128, 512], f32, tag="att",
                                           name=f"att{h}_{j}")
                                 for h in range(2)]
                        for h in range(2):
                            nc.vector.memset(att01[h][:, 0:260], 0.0)
                        st[("att01", j)] = att01
                    r = ki - 4 * j
                    ksl = bass.ts(ki, 128)
                    trim = 128 * r if r >= 0 else 0
                    w = 512 - trim
                    sc = scp.tile([128, 1024], f32, tag="sc", name="sc")
                    for h in range(2):
                        hp = slice(64 * h, 64 * h + 64)
                        nc.tensor.matmul(
                            sc[:, 512 * h + trim:512 * h + 512],
                            kT_AB[hp, ksl],
                            qT_AB[hp, j * 512 + trim:(j + 1) * 512],
                            start=True, stop=True)
                    eb = ebp.tile([128, 1024], bf16, tag="eb", name="eb")
                    sc_ap = sc[:].rearrange(
                        "p (h c) -> p h c", h=2)[:, :, trim:512]
                    eb_ap = eb[:].rearrange(
                        "p (h c) -> p h c", h=2)[:, :, trim:512]
                    used_act = emit_exp([(sc_ap, eb_ap, slice(0, 2 * w))])
                    if 0 <= r < 4:
                        for h in range(2):
                            strip = slice(512 * h + trim,
                                          512 * h + trim + 128)
                            nc.gpsimd.tensor_mul(eb[:, strip], eb[:, strip],
                                                 cmask[:])
                    st[("eb", i)] = eb
                    st[("lag", i)] = 4 if used_act else 6
                else:
                    kp = k
                    if kp == 0:
                        att2 = attp.tile([128, 512], f32, tag="att",
                                         name=f"att2_{j}")
                        nc.vector.memset(att2[:, 0:260], 0.0)
                        st[("att2", j)] = att2
                    sc = scp.tile([128, 1024], f32, tag="sc", name="sc2")
                    eb = ebp.tile([128, 1024], bf16, tag="eb", name="eb2")
                    kis = (2 * kp, 2 * kp + 1)
                    specs = []
                    for half, ki in enumerate(kis):
                        r = ki - 4 * j
                        ksl = bass.ts(ki, 128)
                        trim = 128 * r if r >= 0 else 0
                        nc.tensor.matmul(
                            sc[:, 512 * half + trim:512 * half + 512],
                            kC2[0:64, ksl],
                            qT_C[0:64, j * 512 + trim:(j + 1) * 512],
                            start=True, stop=True)
                        specs.append(
                            (sc[:, 512 * half + trim:512 * half + 512],
                             eb[:, 512 * half + trim:512 * half + 512],
                             slice(512 * half + trim, 512 * half + 512)))
                    if specs[0][2] == slice(0, 512) and \
                       specs[1][2] == slice(512, 1024):
                        specs = [(sc[:], eb[:], slice(0, 1024))]
                    used_act = emit_exp(specs)
                    for half, ki in enumerate(kis):
                        r = ki - 4 * j
                        if 0 <= r < 4:
                            trim = 128 * r
                            strip = slice(512 * half + trim,
                                          512 * half + trim + 128)
                            nc.gpsimd.tensor_mul(eb[:, strip], eb[:, strip],
                                                 cmask[:])
                    st[("eb", i)] = eb
                    st[("lag", i)] = 4 if used_act else 6

            def consume(i):
                kind, j, k = stream[i]
                eb = st.pop(("eb", i))
                st.pop(("lag", i), None)
                if kind == "01":
                    ki = k
                    r = ki - 4 * j
                    att01 = st[("att01", j)]
                    for h in range(2):
                        for c4 in range(4):
                            if r >= 0 and c4 < r:
                                continue
                            nc.tensor.matmul(
                                att01[h][:, c4 * 65:c4 * 65 + 65],
                                eb[:, 512 * h + 128 * c4:
                                   512 * h + 128 * c4 + 128],
                                v_aug[:, ki * 195 + 65 * h:
                                      ki * 195 + 65 * h + 65],
                                start=False, stop=(ki == 4 * j + c4),
                                skip_group_check=True)
                else:
                    kp = k
                    att2 = st[("att2", j)]
                    for half, ki in enumerate((2 * kp, 2 * kp + 1)):
                        r = ki - 4 * j
                        for c4 in range(4):
                            if r >= 0 and c4 < r:
                                continue
                            nc.tensor.matmul(
                                att2[:, c4 * 65:c4 * 65 + 65],
                                eb[:, 512 * half + 128 * c4:
                                   512 * half + 128 * c4 + 128],
                                v_aug[:, ki * 195 + 130:ki * 195 + 195],
                                start=False, stop=(ki == 4 * j + c4),
                                skip_group_check=True)

            def posts(i):
                kind, j, k = stream[i]
                nk = 4 * j + 4
                jsl = bass.ts(j, 512)
                if kind == "01" and k == nk - 1:
                    if dbg and j == 0:
                        att01 = st[("att01", j)]
                        att_t = ebp.tile([128, 512], f32, tag="attt",
                                         name="attt")
                        nc.vector.tensor_copy(att_t[:], att01[0][:])
                        nc.sync.dma_start(dbg_out["d_att"], att_t[:])
                    # normalize + transpose heads 0,1
                    att01 = st.pop(("att01", j))
                    an01 = anp.tile([128, 512], bf16, tag="an01",
                                    name="an01")
                    for h in range(2):
                        rcp = rcpp.tile([128, 4], f32, tag="rcp",
                                        name="rcp")
                        at = att01[h][:]
                        den = bass.AP(at.tensor, at.offset + 64,
                                      [at.ap[0], [65, 4]])
                        nc.vector.reciprocal_approx_fast(out=rcp[:],
                                                         in_=den)
                        for c4 in range(4):
                            nc.vector.tensor_scalar_mul(
                                an01[:, c4 * 128 + 64 * h:
                                     c4 * 128 + 64 * h + 64],
                                att01[h][:, c4 * 65:c4 * 65 + 64],
                                rcp[:, c4:c4 + 1])
                    tr01 = trx_bf[:, 0:512]
                    for c4 in range(4):
                        csl = bass.ts(c4, 128)
                        nc.tensor.transpose(tr01[:, csl], an01[:, csl],
                                            ident[:])
                    nc.vector.tensor_copy(aoT01[:, jsl], tr01)
                elif kind == "2" and k == nk // 2 - 1:
                    # normalize + transpose head 2
                    att2 = st.pop(("att2", j))
                    an2 = anp.tile([128, 256], bf16, tag="an2", name="an2")
                    rcp2 = rcpp.tile([128, 4], f32, tag="rcp", name="rcp2")
                    at2 = att2[:]
                    den2 = bass.AP(at2.tensor, at2.offset + 64,
                                   [at2.ap[0], [65, 4]])
                    nc.vector.reciprocal_approx_fast(out=rcp2[:], in_=den2)
                    for c4 in range(4):
                        nc.vector.tensor_scalar_mul(
                            an2[:, c4 * 64:c4 * 64 + 64],
                            att2[:, c4 * 65:c4 * 65 + 64],
                            rcp2[:, c4:c4 + 1])
                    tr2 = trx_bf[0:64, 0:512]
                    for c4 in range(4):
                        nc.tensor.transpose(tr2[:, bass.ts(c4, 128)],
                                            an2[:, bass.ts(c4, 64)],
                                            ident[:])
                    nc.vector.tensor_copy(aoT2[0:64, jsl], tr2)
                    proj_pending.extend(range(4 * j, 4 * j + 4))

            produce(0)
            if dbg:
                ebt = ebp.tile([128, 1024], f32, tag="ebt", name="ebt")
                nc.vector.tensor_copy(ebt[:], st[("eb", 0)][:])
                nc.sync.dma_start(dbg_out["d_eb"], ebt[:])
            produce(1)
            next_consume = 0

            def drain_due(step, force=False):
                # consume in order every tile whose lag has expired; a tile
                # with unexpired lag blocks later ones (in-order att stop
                # flags within each section stay safe because lag only
                # reorders across exp paths, not the att accumulate order
                # requirement, which is none: start=False adds onto memory)
                nonlocal next_consume
                while next_consume <= step - st.get(("lag", next_consume), 4) \
                        or (force and next_consume < NTILES):
                    if next_consume >= NTILES or ("eb", next_consume) not in st:
                        break
                    i = next_consume
                    next_consume += 1
                    consume(i)
                    posts(i)
                    if i % 2 == 0:
                        emit_proj()

            for step in range(2, NTILES):
                produce(step)
                drain_due(step)
            drain_due(NTILES + 8, force=True)

            while proj_pending:
                emit_proj()
            if dbg:
                t1 = sb.tile([128, T], f32, tag="tao01")
                nc.vector.tensor_copy(t1[:], aoT01[:])
                nc.sync.dma_start(dbg_out["d_ao01"], t1[:])
                t2 = sb.tile([64, T], f32, tag="tao2")
                nc.vector.tensor_copy(t2[0:64, :], aoT2[0:64, :])
                nc.sync.dma_start(dbg_out["d_ao2"], t2[0:64, :])

    nc.compile()
    return nc


_NC_CACHE = {}


def _get_nc(T):
    if T not in _NC_CACHE:
        _NC_CACHE[T] = build_nc(T)
    return _NC_CACHE[T]


def make_core_inputs(x, W_attn, b_attn, W_proj):
    """Host-side prep: per-core input dicts (see module docstring)."""
    B, T, _ = x.shape
    xts = [np.ascontiguousarray(x[b].T).astype(ml_dtypes.bfloat16)
           for b in range(B)]
    # reference splits qkv as (k, q, v)
    Wk, Wq, Wv = W_attn[:, 0:C], W_attn[:, C:2 * C], W_attn[:, 2 * C:3 * C]
    bq_full = b_attn[C:2 * C]
    in_maps = []
    for core in range(N_CORES):
        b = core // (N_CORES // 2)
        h0 = HPC * (core % (N_CORES // 2))
        ccols = slice(h0 * D, (h0 + 2) * D)
        c2 = slice((h0 + 2) * D, (h0 + 3) * D)
        wq = np.concatenate(
            [Wq[:, ccols], Wk[:, ccols], Wq[:, c2], Wk[:, c2],
             Wv[:, h0 * D:(h0 + 3) * D]], axis=1).astype(ml_dtypes.bfloat16)
        bq = np.zeros((128, 2), np.float32)
        bq[:, 0] = bq_full[ccols]
        bq[0:64, 1] = bq_full[c2]
        wp01 = np.ascontiguousarray(
            W_proj[h0 * D:(h0 + 2) * D, :]).astype(ml_dtypes.bfloat16)
        wp2 = np.ascontiguousarray(
            W_proj[(h0 + 2) * D:(h0 + 3) * D, :]).astype(ml_dtypes.bfloat16)
        in_maps.append({"xt": xts[b], "wq": np.ascontiguousarray(wq),
                        "bq": bq, "wp01": wp01, "wp2": wp2})
    return in_maps


def kernel(x, W_attn, b_attn, W_proj, b_proj):
    x = np.asarray(x, dtype=np.float32)
    W_attn = np.asarray(W_attn, dtype=np.float32)
    b_attn = np.asarray(b_attn, dtype=np.float32)
    W_proj = np.asarray(W_proj, dtype=np.float32)
    b_proj = np.asarray(b_proj, dtype=np.float32)
    B, T, _ = x.shape

    nc = _get_nc(T)
    in_maps = make_core_inputs(x, W_attn, b_attn, W_proj)
    res = None
    for attempt in range(3):
        try:
            res = run_bass_kernel_spmd(nc, in_maps, list(range(N_CORES)))
            break
        except Exception:
            if attempt == 2:
                raise
    global LAST_RUN
    LAST_RUN = res

    gpb = N_CORES // B
    # v-bias folded: softmax weights sum to 1 per row
    b_eff = b_proj + b_attn[2 * C:3 * C] @ W_proj
    out = np.empty((B, T, C), np.float32)
    for b in range(B):
        acc = res.results[b * gpb]["y"].astype(np.float32)
        for g in range(1, gpb):
            acc = acc + res.results[b * gpb + g]["y"]
        out[b] = acc + b_eff[None, :]
    return out


# revision 30
# speedup vs baseline: 1.1419x; 1.1419x over previous
"""Causal self-attention (B=2, T=4096, C=768, H=12) on 8 TRN2 NeuronCores.

Sharding: batch x head-group. Core c handles batch b=c//4 and heads
h0..h0+2 where h0 = 3*(c%4). Each core computes the qkv projection for
its 3 heads, full causal attention, and a partial output projection; the
host sums the 4 partials per batch and adds the (augmented) projection
bias.

Numerics / structure:
- All matmul operands are bf16 (f32 PSUM accumulation).
- k-bias is dropped entirely: softmax over k is invariant to the
  per-query constant q . b_k. v-bias is folded into the host-side output
  bias (softmax weights sum to 1, so + b_v @ W_proj).
- q/k live transposed ([d, T]) feeding the scores matmul; v is computed
  directly in natural layout ([T, d]) with an interleaved ones column
  per head providing the softmax denominator through the att@v matmul.
- scores^T tiles [128 k, 512 q] -> exp -> eb (bf16). exp runs on the Act
  engine for ~3/4 of tiles and as a Schraudolph int16-bitcast
  approximation (tensor_scalar mult+add, then f32->int16 convert copy)
  on DVE for the rest, keeping both engines busy.
- att@v is "flipped": out [128 q, 65] accumulated over k-tiles (65
  cycles per matmul instead of 512), using eb chunks as the stationary
  operand. PSUM accumulation uses pre-zeroed banks + start=False
  matmuls: a start=True wipes the whole bank's in-flight accumulation,
  so interleaved per-chunk groups in one bank need memory-accumulate.
  Diagonal tiles skip fully-masked chunks/columns.
- Normalization uses the denominator column as a per-partition scalar
  (reciprocal + tensor_scalar_mul), then PE transposes the normalized
  [q, d] chunks to d-major for the output projection.
- Phase 3 is software-pipelined: scores/exp for tile ki are emitted one
  step ahead of the att@v consuming ki-1, so att@v sem-waits (4-deep PE
  wait station) never block the next tile's score matmuls.
"""

import sys

for _p in ("/opt/trn_rl_repo",):
    if _p not in sys.path:
        sys.path.insert(0, _p)

import math
import os
from contextlib import ExitStack

import numpy as np
import ml_dtypes

import concourse.bass as bass
import concourse.mybir as mybir
import concourse.tile as tile
from concourse import bacc
from concourse.bass_utils import run_bass_kernel_spmd
from concourse.masks import make_identity

f32 = mybir.dt.float32
bf16 = mybir.dt.bfloat16
i16 = mybir.dt.int16
AF = mybir.ActivationFunctionType
Alu = mybir.AluOpType

C = 768
D = 64
HPC = 3  # heads per core
N_CORES = 8
CK = C // 128  # contraction chunks

SCALE = 1.0 / math.sqrt(D)  # 0.125
# Schraudolph: exp(s*SCALE) ~ bitcast_bf16(int16(s*A_SCH + B_SCH))
A_SCH = SCALE * 128.0 / math.log(2.0)
B_SCH = 128.0 * (127.0 - 0.043)

# exp path pattern over score tiles: True -> Act exp, False -> DVE
# Schraudolph (2 DVE ops).
EXP_PAT = [True, True, True, False, True, True, True, True, False]


def build_nc(T):
    NT = T // 512  # q tiles
    KT = T // 128  # k tiles
    T2 = T // 2

    nc = bacc.Bacc("TRN2", target_bir_lowering=False, debug=False,
                   num_devices=N_CORES)
    xt_d = nc.dram_tensor("xt", [C, T], bf16, kind="ExternalInput").ap()
    wq_d = nc.dram_tensor("wq", [C, 576], bf16, kind="ExternalInput").ap()
    bq_d = nc.dram_tensor("bq", [128, 2], f32, kind="ExternalInput").ap()
    wp01_d = nc.dram_tensor("wp01", [128, C], bf16, kind="ExternalInput").ap()
    wp2_d = nc.dram_tensor("wp2", [64, C], bf16, kind="ExternalInput").ap()
    y_d = nc.dram_tensor("y", [T, C], f32, kind="ExternalOutput").ap()
    dbg = os.environ.get("KDBG") == "1"
    dbg_out = {}
    if dbg:
        for nm, shp in [("d_qAB", [128, T]), ("d_kAB", [128, T]),
                        ("d_qC", [64, T]), ("d_kC2", [64, T]),
                        ("d_vaug", [128, KT * 195]),
                        ("d_ao01", [128, T]), ("d_ao2", [64, T]),
                        ("d_eb", [128, 1024]), ("d_att", [128, 512])]:
            dbg_out[nm] = nc.dram_tensor(nm, shp, f32, kind="ExternalOutput").ap()

    with tile.TileContext(nc) as tc, ExitStack() as ctx:
        sb = ctx.enter_context(tc.tile_pool(name="sb", bufs=1))

        # persistent tensors
        bq_sb = sb.tile([128, 2], f32, tag="bq")
        qT_AB = sb.tile([128, T], bf16, tag="qAB")
        kT_AB = sb.tile([128, T], bf16, tag="kAB")
        qT_C = sb.tile([64, T], bf16, tag="qC")
        kC2 = sb.tile([64, T], bf16, tag="kC2")
        v_aug = sb.tile([128, KT * 195], bf16, tag="vaug")
        aoT01 = sb.tile([128, T], bf16, tag="aoT01")
        aoT2 = sb.tile([64, T], bf16, tag="aoT2")
        ident = sb.tile([128, 128], bf16, tag="ident")
        cmask = sb.tile([128, 128], bf16, tag="cmask")

        nc.sync.dma_start(bq_sb[:], bq_d)
        make_identity(nc, ident[:])
        # causal triangle for the in-diagonal 128-col strip: keep col >= part
        nc.gpsimd.memset(cmask[:], 1.0)
        nc.gpsimd.affine_select(
            cmask[:], cmask[:], pattern=[[1, 128]],
            compare_op=Alu.is_ge, fill=0.0, base=0, channel_multiplier=-1)
        # ones columns of v_aug (denominator rows through att@v)
        ones_cols = v_aug[:].rearrange("p (k h c) -> p k h c", h=3, c=65)[:, :, :, 64:65]
        nc.gpsimd.memset(ones_cols, 1.0)

        wpp = ctx.enter_context(tc.tile_pool(name="wpp", bufs=1))
        wp01 = wpp.tile([128, C], bf16, tag="wp01")
        wp2 = wpp.tile([64, C], bf16, tag="wp2")

        es_p1 = ExitStack()
        xtp = es_p1.enter_context(tc.tile_pool(name="xtp", bufs=1))
        wqp = es_p1.enter_context(tc.tile_pool(name="wqp", bufs=1))
        k2p = es_p1.enter_context(tc.tile_pool(name="k2p", bufs=1))
        xt_sb = [xtp.tile([128, T], bf16, tag=f"xt{c}", name=f"xt{c}")
                 for c in range(CK)]
        wq_sb = [wqp.tile([128, 576], bf16, tag=f"wq{c}", name=f"wq{c}")
                 for c in range(CK)]
        k2s = k2p.tile([128, T], bf16, tag="k2s")
        for c in range(CK):
            nc.sync.dma_start(wq_sb[c][:], wq_d[c * 128:(c + 1) * 128, :])
        for c in range(CK):
            nc.sync.dma_start(xt_sb[c][:, 0:512],
                              xt_d[c * 128:(c + 1) * 128, 0:512])
        for h0_, h1_ in ((512, T2), (T2, T)):
            if h1_ <= h0_:
                continue
            for c in range(CK):
                nc.sync.dma_start(xt_sb[c][:, h0_:h1_],
                                  xt_d[c * 128:(c + 1) * 128, h0_:h1_])
        nc.sync.dma_start(wp01[:], wp01_d)
        nc.sync.dma_start(wp2[:], wp2_d)

        # ---------------- phase 1: qkv projection --------------------------
        with tc.tile_pool(name="qkp", bufs=2, space="PSUM") as qkp, \
             tc.tile_pool(name="vps", bufs=2, space="PSUM") as vps:
            for j in range(NT):
                jsl = bass.ts(j, 512)
                q01 = qkp.tile([128, 512], f32, tag="q01", name="q01")
                k01 = qkp.tile([128, 512], f32, tag="k01", name="k01")
                qk2 = qkp.tile([128, 512], f32, tag="qk2", name="qk2")
                for c in range(CK):
                    st, sp = c == 0, c == CK - 1
                    nc.tensor.matmul(q01[:], wq_sb[c][:, 0:128],
                                     xt_sb[c][:, jsl], start=st, stop=sp)
                    nc.tensor.matmul(k01[:], wq_sb[c][:, 128:256],
                                     xt_sb[c][:, jsl], start=st, stop=sp)
                    nc.tensor.matmul(qk2[:], wq_sb[c][:, 256:384],
                                     xt_sb[c][:, jsl], start=st, stop=sp)
                nc.vector.tensor_scalar_add(qT_AB[:, jsl], q01[:],
                                            bq_sb[:, 0:1])
                nc.vector.tensor_copy(kT_AB[:, jsl], k01[:])
                nc.vector.tensor_scalar_add(qT_C[0:64, jsl], qk2[0:64, :],
                                            bq_sb[0:64, 1:2])
                nc.vector.tensor_copy(k2s[64:128, jsl], qk2[64:128, :])
                for mi in range(4):
                    m = 4 * j + mi
                    msl = bass.ts(m, 128)
                    vp = vps.tile([128, 192], f32, tag="vp", name="vp")
                    for c in range(CK):
                        nc.tensor.matmul(vp[:], xt_sb[c][:, msl],
                                         wq_sb[c][:, 384:576],
                                         start=(c == 0), stop=(c == CK - 1))
                    vdst = v_aug[:, m * 195:(m + 1) * 195].rearrange(
                        "p (h c) -> p h c", c=65)[:, :, 0:64]
                    nc.vector.tensor_copy(
                        vdst, vp[:].rearrange("p (h c) -> p h c", c=64))
        # k2: partition shift 64:128 -> 0:64 via sbuf->sbuf DMA
        nc.sync.dma_start(kC2[0:64, :], k2s[64:128, :])
        es_p1.close()
        if dbg:
            for nm, t_ in [("d_qAB", qT_AB), ("d_kAB", kT_AB),
                           ("d_vaug", v_aug)]:
                tmp = sb.tile(list(t_.shape), f32, tag=f"t{nm}", name=f"t{nm}")
                nc.vector.tensor_copy(tmp[:], t_[:])
                nc.sync.dma_start(dbg_out[nm], tmp[:])
            for nm, t_ in [("d_qC", qT_C), ("d_kC2", kC2)]:
                tmp = sb.tile([64, T], f32, tag=f"t{nm}", name=f"t{nm}")
                nc.vector.tensor_copy(tmp[0:64, :], t_[0:64, :])
                nc.sync.dma_start(dbg_out[nm], tmp[0:64, :])

        # ---------------- phase 3: attention + projection ------------------
        with tc.tile_pool(name="scp", bufs=2, space="PSUM") as scp, \
             tc.tile_pool(name="attp", bufs=2, space="PSUM") as attp, \
             tc.tile_pool(name="trp", bufs=1, space="PSUM") as trp, \
             tc.tile_pool(name="ebp", bufs=7) as ebp, \
             tc.tile_pool(name="ebfp", bufs=3) as ebfp, \
             tc.tile_pool(name="anp", bufs=2) as anp, \
             tc.tile_pool(name="rcpp", bufs=6) as rcpp, \
             tc.tile_pool(name="ysp", bufs=4) as ysp:

            # 2 banks of manually sub-allocated scratch: transposes (bf16
            # view of f32 cols 0:256, h01/h2 sequential) + projection psum
            # thirds (pyA f32 cols 256:512, pyB 512:768). Tile dep-tracking
            # is range-precise, so disjoint sub-ranges don't serialize.
            trx = trp.tile([128, 1024], f32, tag="trx", name="trx")
            trx_bf = trx[:, 0:256].bitcast(bf16)  # [128, 512] bf16
            exp_ctr = [0]

            def emit_exp(specs):
                """specs: list of (sc_ap, eb_ap, ebf_cols) with matching
                shapes; one score tile, routed to Act or DVE. Returns True
                if the Act path was used."""
                use_act = EXP_PAT[exp_ctr[0] % len(EXP_PAT)]
                exp_ctr[0] += 1
                if use_act:
                    for sc_ap, eb_ap, _ in specs:
                        nc.scalar.activation(eb_ap, sc_ap, AF.Exp,
                                             scale=SCALE)
                else:
                    ebf = ebfp.tile([128, 1024], f32, tag="ebf", name="ebf")
                    for sc_ap, eb_ap, cols in specs:
                        ebf_ap = ebf[:, cols]
                        if len(sc_ap.shape) == 3:
                            ebf_ap = ebf_ap.rearrange(
                                "p (h c) -> p h c", h=sc_ap.shape[1])
                        nc.vector.tensor_scalar(ebf_ap, sc_ap,
                                                A_SCH, B_SCH,
                                                Alu.mult, Alu.add)
                        nc.vector.tensor_copy(eb_ap.bitcast(i16), ebf_ap)
                return use_act

            proj_pending = []
            py_ctr = [0]

            def emit_proj():
                if not proj_pending:
                    return
                m = proj_pending.pop(0)
                msl = bass.ts(m, 128)
                y_sb = ysp.tile([128, C], f32, tag="ysb", name="ysb")
                for third in range(3):
                    off = 256 + 256 * (py_ctr[0] % 2)
                    py_ctr[0] += 1
                    py = trx[:, off:off + 256]
                    csl = slice(third * 256, (third + 1) * 256)
                    nc.tensor.matmul(py, aoT01[:, msl], wp01[:, csl],
                                     start=True, stop=False,
                                     skip_group_check=True)
                    nc.tensor.matmul(py, aoT2[0:64, msl], wp2[0:64, csl],
                                     start=False, stop=True,
                                     skip_group_check=True)
                    nc.vector.tensor_copy(y_sb[:, csl], py)
                nc.sync.dma_start(y_d[m * 128:(m + 1) * 128, :], y_sb[:])

            # ---- flat software-pipelined tile stream across all j,
            # heads 0,1 (per k-tile) and head 2 (per k-tile pair).
            # Emission order per step: consume(i) -> posts(i) ->
            # produce(i+2), keeping 2 score tiles in flight so att@v never
            # waits on exp and PSUM slot reuse (WAR) never blocks.
            stream = []
            for j in range(NT):
                nk = 4 * j + 4
                for ki in range(nk):
                    stream.append(("01", j, ki))
                for kp in range(nk // 2):
                    stream.append(("2", j, kp))
            NTILES = len(stream)

            st = {}  # per-j live tiles: att01, att2, eb tiles

            def produce(i):
                kind, j, k = stream[i]
                nk = 4 * j + 4
                if kind == "01":
                    ki = k
                    if ki == 0:
                        att01 = [attp.tile([128, 512], f32, tag="att",
                                           name=f"att{h}_{j}")
                                 for h in range(2)]
                        for h in range(2):
                            nc.vector.memset(att01[h][:, 0:260], 0.0)
                        st[("att01", j)] = att01
                    r = ki - 4 * j
                    ksl = bass.ts(ki, 128)
                    trim = 128 * r if r >= 0 else 0
                    w = 512 - trim
                    sc = scp.tile([128, 1024], f32, tag="sc", name="sc")
                    for h in range(2):
                        hp = slice(64 * h, 64 * h + 64)
                        nc.tensor.matmul(
                            sc[:, 512 * h + trim:512 * h + 512],
                            kT_AB[hp, ksl],
                            qT_AB[hp, j * 512 + trim:(j + 1) * 512],
                            start=True, stop=True)
                    eb = ebp.tile([128, 1024], bf16, tag="eb", name="eb")
                    sc_ap = sc[:].rearrange(
                        "p (h c) -> p h c", h=2)[:, :, trim:512]
                    eb_ap = eb[:].rearrange(
                        "p (h c) -> p h c", h=2)[:, :, trim:512]
                    used_act = emit_exp([(sc_ap, eb_ap, slice(0, 2 * w))])
                    if 0 <= r < 4:
                        for h in range(2):
                            strip = slice(512 * h + trim,
                                          512 * h + trim + 128)
                            nc.gpsimd.tensor_mul(eb[:, strip], eb[:, strip],
                                                 cmask[:])
                    st[("eb", i)] = eb
                    st[("lag", i)] = 4 if used_act else 6
                else:
                    kp = k
                    if kp == 0:
                        att2 = attp.tile([128, 512], f32, tag="att",
                                         name=f"att2_{j}")
                        nc.vector.memset(att2[:, 0:260], 0.0)
                        st[("att2", j)] = att2
                    sc = scp.tile([128, 1024], f32, tag="sc", name="sc2")
                    eb = ebp.tile([128, 1024], bf16, tag="eb", name="eb2")
                    kis = (2 * kp, 2 * kp + 1)
                    specs = []
                    for half, ki in enumerate(kis):
                        r = ki - 4 * j
                        ksl = bass.ts(ki, 128)
                        trim = 128 * r if r >= 0 else 0
                        nc.tensor.matmul(
                            sc[:, 512 * half + trim:512 * half + 512],
                            kC2[0:64, ksl],
                            qT_C[0:64, j * 512 + trim:(j + 1) * 512],
                            start=True, stop=True)
                        specs.append(
                            (sc[:, 512 * half + trim:512 * half + 512],
                             eb[:, 512 * half + trim:512 * half + 512],
                             slice(512 * half + trim, 512 * half + 512)))
                    if specs[0][2] == slice(0, 512) and \
                       specs[1][2] == slice(512, 1024):
                        specs = [(sc[:], eb[:], slice(0, 1024))]
                    used_act = emit_exp(specs)
                    for half, ki in enumerate(kis):
                        r = ki - 4 * j
                        if 0 <= r < 4:
                            trim = 128 * r
                            strip = slice(512 * half + trim,
                                          512 * half + trim + 128)
                            nc.gpsimd.tensor_mul(eb[:, strip], eb[:, strip],
                                                 cmask[:])
                    st[("eb", i)] = eb
                    st[("lag", i)] = 4 if used_act else 6

            def consume(i):
                kind, j, k = stream[i]
                eb = st.pop(("eb", i))
                st.pop(("lag", i), None)
                if kind == "01":
                    ki = k
                    r = ki - 4 * j
                    att01 = st[("att01", j)]
                    for h in range(2):
                        for c4 in range(4):
                            if r >= 0 and c4 < r:
                                continue
                            nc.tensor.matmul(
                                att01[h][:, c4 * 65:c4 * 65 + 65],
                                eb[:, 512 * h + 128 * c4:
                                   512 * h + 128 * c4 + 128],
                                v_aug[:, ki * 195 + 65 * h:
                                      ki * 195 + 65 * h + 65],
                                start=False, stop=(ki == 4 * j + c4),
                                skip_group_check=True)
                else:
                    kp = k
                    att2 = st[("att2", j)]
                    for half, ki in enumerate((2 * kp, 2 * kp + 1)):
                        r = ki - 4 * j
                        for c4 in range(4):
                            if r >= 0 and c4 < r:
                                continue
                            nc.tensor.matmul(
                                att2[:, c4 * 65:c4 * 65 + 65],
                                eb[:, 512 * half + 128 * c4:
                                   512 * half + 128 * c4 + 128],
                                v_aug[:, ki * 195 + 130:ki * 195 + 195],
                                start=False, stop=(ki == 4 * j + c4),
                                skip_group_check=True)

            def posts(i):
                kind, j, k = stream[i]
                nk = 4 * j + 4
                jsl = bass.ts(j, 512)
                if kind == "01" and k == nk - 1:
                    if dbg and j == 0:
                        att01 = st[("att01", j)]
                        att_t = ebp.tile([128, 512], f32, tag="attt",
                                         name="attt")
                        nc.vector.tensor_copy(att_t[:], att01[0][:])
                        nc.sync.dma_start(dbg_out["d_att"], att_t[:])
                    # normalize + transpose heads 0,1
                    att01 = st.pop(("att01", j))
                    an01 = anp.tile([128, 512], bf16, tag="an01",
                                    name="an01")
                    for h in range(2):
                        rcp = rcpp.tile([128, 4], f32, tag="rcp",
                                        name="rcp")
                        at = att01[h][:]
                        den = bass.AP(at.tensor, at.offset + 64,
                                      [at.ap[0], [65, 4]])
                        nc.vector.reciprocal_approx_fast(out=rcp[:],
                                                         in_=den)
                        for c4 in range(4):
                            nc.vector.tensor_scalar_mul(
                                an01[:, c4 * 128 + 64 * h:
                                     c4 * 128 + 64 * h + 64],
                                att01[h][:, c4 * 65:c4 * 65 + 64],
                                rcp[:, c4:c4 + 1])
                    tr01 = trx_bf[:, 0:512]
                    for c4 in range(4):
                        csl = bass.ts(c4, 128)
                        nc.tensor.transpose(tr01[:, csl], an01[:, csl],
                                            ident[:])
                    nc.vector.tensor_copy(aoT01[:, jsl], tr01)
                elif kind == "2" and k == nk // 2 - 1:
                    # normalize + transpose head 2
                    att2 = st.pop(("att2", j))
                    an2 = anp.tile([128, 256], bf16, tag="an2", name="an2")
                    rcp2 = rcpp.tile([128, 4], f32, tag="rcp", name="rcp2")
                    at2 = att2[:]
                    den2 = bass.AP(at2.tensor, at2.offset + 64,
                                   [at2.ap[0], [65, 4]])
                    nc.vector.reciprocal_approx_fast(out=rcp2[:], in_=den2)
                    for c4 in range(4):
                        nc.vector.tensor_scalar_mul(
                            an2[:, c4 * 64:c4 * 64 + 64],
                            att2[:, c4 * 65:c4 * 65 + 64],
                            rcp2[:, c4:c4 + 1])
                    tr2 = trx_bf[0:64, 0:512]
                    for c4 in range(4):
                        nc.tensor.transpose(tr2[:, bass.ts(c4, 128)],
                                            an2[:, bass.ts(c4, 64)],
                                            ident[:])
                    nc.vector.tensor_copy(aoT2[0:64, jsl], tr2)
                    proj_pending.extend(range(4 * j, 4 * j + 4))

            produce(0)
            if dbg:
                ebt = ebp.tile([128, 1024], f32, tag="ebt", name="ebt")
                nc.vector.tensor_copy(ebt[:], st[("eb", 0)][:])
                nc.sync.dma_start(dbg_out["d_eb"], ebt[:])
            produce(1)
            next_consume = 0

            def drain_due(step, force=False):
                # consume in order every tile whose lag has expired; a tile
                # with unexpired lag blocks later ones (in-order att stop
                # flags within each section stay safe because lag only
                # reorders across exp paths, not the att accumulate order
                # requirement, which is none: start=False adds onto memory)
                nonlocal next_consume
                while next_consume <= step - st.get(("lag", next_consume), 4) \
                        or (force and next_consume < NTILES):
                    if next_consume >= NTILES or ("eb", next_consume) not in st:
                        break
                    i = next_consume
                    next_consume += 1
                    consume(i)
                    posts(i)
                    if i % 2 == 0:
                        emit_proj()

            for step in range(2, NTILES):
                produce(step)
                drain_due(step)
            drain_due(NTILES + 8, force=True)

            while proj_pending:
                emit_proj()
            if dbg:
                t1 = sb.tile([128, T], f32, tag="tao01")
                nc.vector.tensor_copy(t1[:], aoT01[:])
                nc.sync.dma_start(dbg_out["d_ao01"], t1[:])
                t2 = sb.tile([64, T], f32, tag="tao2")
                nc.vector.tensor_copy(t2[0:64, :], aoT2[0:64, :])
                nc.sync.dma_start(dbg_out["d_ao2"], t2[0:64, :])

    nc.compile()
    return nc


_NC_CACHE = {}


def _get_nc(T):
    if T not in _NC_CACHE:
        _NC_CACHE[T] = build_nc(T)
    return _NC_CACHE[T]


def make_core_inputs(x, W_attn, b_attn, W_proj):
    """Host-side prep: per-core input dicts (see module docstring)."""
    B, T, _ = x.shape
    xts = [np.ascontiguousarray(x[b].T).astype(ml_dtypes.bfloat16)
           for b in range(B)]
    # reference splits qkv as (k, q, v)
    Wk, Wq, Wv = W_attn[:, 0:C], W_attn[:, C:2 * C], W_attn[:, 2 * C:3 * C]
    bq_full = b_attn[C:2 * C]
    in_maps = []
    for core in range(N_CORES):
        b = core // (N_CORES // 2)
        h0 = HPC * (core % (N_CORES // 2))
        ccols = slice(h0 * D, (h0 + 2) * D)
        c2 = slice((h0 + 2) * D, (h0 + 3) * D)
        wq = np.concatenate(
            [Wq[:, ccols], Wk[:, ccols], Wq[:, c2], Wk[:, c2],
             Wv[:, h0 * D:(h0 + 3) * D]], axis=1).astype(ml_dtypes.bfloat16)
        bq = np.zeros((128, 2), np.float32)
        bq[:, 0] = bq_full[ccols]
        bq[0:64, 1] = bq_full[c2]
        wp01 = np.ascontiguousarray(
            W_proj[h0 * D:(h0 + 2) * D, :]).astype(ml_dtypes.bfloat16)
        wp2 = np.ascontiguousarray(
            W_proj[(h0 + 2) * D:(h0 + 3) * D, :]).astype(ml_dtypes.bfloat16)
        in_maps.append({"xt": xts[b], "wq": np.ascontiguousarray(wq),
                        "bq": bq, "wp01": wp01, "wp2": wp2})
    return in_maps


def kernel(x, W_attn, b_attn, W_proj, b_proj):
    x = np.asarray(x, dtype=np.float32)
    W_attn = np.asarray(W_attn, dtype=np.float32)
    b_attn = np.asarray(b_attn, dtype=np.float32)
    W_proj = np.asarray(W_proj, dtype=np.float32)
    b_proj = np.asarray(b_proj, dtype=np.float32)
    B, T, _ = x.shape

    nc = _get_nc(T)
    in_maps = make_core_inputs(x, W_attn, b_attn, W_proj)
    res = None
    for attempt in range(3):
        try:
            res = run_bass_kernel_spmd(nc, in_maps, list(range(N_CORES)))
            break
        except Exception:
            if attempt == 2:
                raise
    global LAST_RUN
    LAST_RUN = res

    gpb = N_CORES // B
    # v-bias folded: softmax weights sum to 1 per row
    b_eff = b_proj + b_attn[2 * C:3 * C] @ W_proj
    out = np.empty((B, T, C), np.float32)
    for b in range(B):
        acc = res.results[b * gpb]["y"].astype(np.float32)
        for g in range(1, gpb):
            acc = acc + res.results[b * gpb + g]["y"]
        out[b] = acc + b_eff[None, :]
    return out


# revision 32
# speedup vs baseline: 1.1541x; 1.0107x over previous
"""Causal self-attention (B=2, T=4096, C=768, H=12) on 8 TRN2 NeuronCores.

Sharding: batch x head-group. Core c handles batch b=c//4 and heads
h0..h0+2 where h0 = 3*(c%4). Each core computes the qkv projection for
its 3 heads, full causal attention, and a partial output projection; the
host sums the 4 partials per batch and adds the (augmented) projection
bias.

Numerics / structure:
- All matmul operands are bf16 (f32 PSUM accumulation).
- k-bias is dropped entirely: softmax over k is invariant to the
  per-query constant q . b_k. v-bias is folded into the host-side output
  bias (softmax weights sum to 1, so + b_v @ W_proj).
- q/k live transposed ([d, T]) feeding the scores matmul; v is computed
  directly in natural layout ([T, d]) with an interleaved ones column
  per head providing the softmax denominator through the att@v matmul.
- scores^T tiles [128 k, 512 q] -> exp -> eb (bf16). exp runs on the Act
  engine for ~3/4 of tiles and as a Schraudolph int16-bitcast
  approximation (tensor_scalar mult+add, then f32->int16 convert copy)
  on DVE for the rest, keeping both engines busy.
- att@v is "flipped": out [128 q, 65] accumulated over k-tiles (65
  cycles per matmul instead of 512), using eb chunks as the stationary
  operand. PSUM accumulation uses pre-zeroed banks + start=False
  matmuls: a start=True wipes the whole bank's in-flight accumulation,
  so interleaved per-chunk groups in one bank need memory-accumulate.
  Diagonal tiles skip fully-masked chunks/columns.
- Normalization uses the denominator column as a per-partition scalar
  (reciprocal + tensor_scalar_mul), then PE transposes the normalized
  [q, d] chunks to d-major for the output projection.
- Phase 3 is software-pipelined: scores/exp for tile ki are emitted one
  step ahead of the att@v consuming ki-1, so att@v sem-waits (4-deep PE
  wait station) never block the next tile's score matmuls.
"""

import sys

for _p in ("/opt/trn_rl_repo",):
    if _p not in sys.path:
        sys.path.insert(0, _p)

import math
import os
from contextlib import ExitStack

import numpy as np
import ml_dtypes

import concourse.bass as bass
import concourse.mybir as mybir
import concourse.tile as tile
from concourse import bacc
from concourse.bass_utils import run_bass_kernel_spmd
from concourse.masks import make_identity

f32 = mybir.dt.float32
bf16 = mybir.dt.bfloat16
i16 = mybir.dt.int16
AF = mybir.ActivationFunctionType
Alu = mybir.AluOpType

C = 768
D = 64
HPC = 3  # heads per core
N_CORES = 8
CK = C // 128  # contraction chunks

SCALE = 1.0 / math.sqrt(D)  # 0.125
# Schraudolph: exp(s*SCALE) ~ bitcast_bf16(int16(s*A_SCH + B_SCH))
A_SCH = SCALE * 128.0 / math.log(2.0)
B_SCH = 128.0 * (127.0 - 0.043)

# exp path pattern over score tiles: True -> Act exp, False -> DVE
# Schraudolph (2 DVE ops).
EXP_PAT = [True, True, True, True, False]


def build_nc(T):
    NT = T // 512  # q tiles
    KT = T // 128  # k tiles
    T2 = T // 2

    nc = bacc.Bacc("TRN2", target_bir_lowering=False, debug=False,
                   num_devices=N_CORES)
    xt_d = nc.dram_tensor("xt", [C, T], bf16, kind="ExternalInput").ap()
    wq_d = nc.dram_tensor("wq", [C, 576], bf16, kind="ExternalInput").ap()
    bq_d = nc.dram_tensor("bq", [128, 2], f32, kind="ExternalInput").ap()
    wp01_d = nc.dram_tensor("wp01", [128, C], bf16, kind="ExternalInput").ap()
    wp2_d = nc.dram_tensor("wp2", [64, C], bf16, kind="ExternalInput").ap()
    y_d = nc.dram_tensor("y", [T, C], f32, kind="ExternalOutput").ap()
    dbg = os.environ.get("KDBG") == "1"
    dbg_out = {}
    if dbg:
        for nm, shp in [("d_qAB", [128, T]), ("d_kAB", [128, T]),
                        ("d_qC", [64, T]), ("d_kC2", [64, T]),
                        ("d_vaug", [128, KT * 195]),
                        ("d_ao01", [128, T]), ("d_ao2", [64, T]),
                        ("d_eb", [128, 1024]), ("d_att", [128, 512])]:
            dbg_out[nm] = nc.dram_tensor(nm, shp, f32, kind="ExternalOutput").ap()

    with tile.TileContext(nc) as tc, ExitStack() as ctx:
        sb = ctx.enter_context(tc.tile_pool(name="sb", bufs=1))

        # persistent tensors
        bq_sb = sb.tile([128, 2], f32, tag="bq")
        qT_AB = sb.tile([128, T], bf16, tag="qAB")
        kT_AB = sb.tile([128, T], bf16, tag="kAB")
        qT_C = sb.tile([64, T], bf16, tag="qC")
        kC2 = sb.tile([64, T], bf16, tag="kC2")
        v_aug = sb.tile([128, KT * 195], bf16, tag="vaug")
        aoT01 = sb.tile([128, T], bf16, tag="aoT01")
        aoT2 = sb.tile([64, T], bf16, tag="aoT2")
        ident = sb.tile([128, 128], bf16, tag="ident")
        cmask = sb.tile([128, 128], bf16, tag="cmask")

        nc.sync.dma_start(bq_sb[:], bq_d)
        make_identity(nc, ident[:])
        # causal triangle for the in-diagonal 128-col strip: keep col >= part
        nc.gpsimd.memset(cmask[:], 1.0)
        nc.gpsimd.affine_select(
            cmask[:], cmask[:], pattern=[[1, 128]],
            compare_op=Alu.is_ge, fill=0.0, base=0, channel_multiplier=-1)
        # ones columns of v_aug (denominator rows through att@v)
        ones_cols = v_aug[:].rearrange("p (k h c) -> p k h c", h=3, c=65)[:, :, :, 64:65]
        nc.gpsimd.memset(ones_cols, 1.0)

        wpp = ctx.enter_context(tc.tile_pool(name="wpp", bufs=1))
        wp01 = wpp.tile([128, C], bf16, tag="wp01")
        wp2 = wpp.tile([64, C], bf16, tag="wp2")

        es_p1 = ExitStack()
        xtp = es_p1.enter_context(tc.tile_pool(name="xtp", bufs=1))
        wqp = es_p1.enter_context(tc.tile_pool(name="wqp", bufs=1))
        k2p = es_p1.enter_context(tc.tile_pool(name="k2p", bufs=1))
        xt_sb = [xtp.tile([128, T], bf16, tag=f"xt{c}", name=f"xt{c}")
                 for c in range(CK)]
        wq_sb = [wqp.tile([128, 576], bf16, tag=f"wq{c}", name=f"wq{c}")
                 for c in range(CK)]
        k2s = k2p.tile([128, T], bf16, tag="k2s")
        for c in range(CK):
            nc.sync.dma_start(wq_sb[c][:], wq_d[c * 128:(c + 1) * 128, :])
        for c in range(CK):
            nc.sync.dma_start(xt_sb[c][:, 0:512],
                              xt_d[c * 128:(c + 1) * 128, 0:512])
        for h0_, h1_ in ((512, T2), (T2, T)):
            if h1_ <= h0_:
                continue
            for c in range(CK):
                nc.sync.dma_start(xt_sb[c][:, h0_:h1_],
                                  xt_d[c * 128:(c + 1) * 128, h0_:h1_])
        nc.sync.dma_start(wp01[:], wp01_d)
        nc.sync.dma_start(wp2[:], wp2_d)

        # ---------------- phase 1: qkv projection --------------------------
        with tc.tile_pool(name="qkp", bufs=2, space="PSUM") as qkp, \
             tc.tile_pool(name="vps", bufs=2, space="PSUM") as vps:
            for j in range(NT):
                jsl = bass.ts(j, 512)
                q01 = qkp.tile([128, 512], f32, tag="q01", name="q01")
                k01 = qkp.tile([128, 512], f32, tag="k01", name="k01")
                qk2 = qkp.tile([128, 512], f32, tag="qk2", name="qk2")
                for c in range(CK):
                    st, sp = c == 0, c == CK - 1
                    nc.tensor.matmul(q01[:], wq_sb[c][:, 0:128],
                                     xt_sb[c][:, jsl], start=st, stop=sp)
                    nc.tensor.matmul(k01[:], wq_sb[c][:, 128:256],
                                     xt_sb[c][:, jsl], start=st, stop=sp)
                    nc.tensor.matmul(qk2[:], wq_sb[c][:, 256:384],
                                     xt_sb[c][:, jsl], start=st, stop=sp)
                nc.vector.tensor_scalar_add(qT_AB[:, jsl], q01[:],
                                            bq_sb[:, 0:1])
                nc.vector.tensor_copy(kT_AB[:, jsl], k01[:])
                nc.vector.tensor_scalar_add(qT_C[0:64, jsl], qk2[0:64, :],
                                            bq_sb[0:64, 1:2])
                nc.vector.tensor_copy(k2s[64:128, jsl], qk2[64:128, :])
                for mi in range(4):
                    m = 4 * j + mi
                    msl = bass.ts(m, 128)
                    vp = vps.tile([128, 192], f32, tag="vp", name="vp")
                    for c in range(CK):
                        nc.tensor.matmul(vp[:], xt_sb[c][:, msl],
                                         wq_sb[c][:, 384:576],
                                         start=(c == 0), stop=(c == CK - 1))
                    vdst = v_aug[:, m * 195:(m + 1) * 195].rearrange(
                        "p (h c) -> p h c", c=65)[:, :, 0:64]
                    nc.vector.tensor_copy(
                        vdst, vp[:].rearrange("p (h c) -> p h c", c=64))
        # k2: partition shift 64:128 -> 0:64 via sbuf->sbuf DMA
        nc.sync.dma_start(kC2[0:64, :], k2s[64:128, :])
        es_p1.close()
        if dbg:
            for nm, t_ in [("d_qAB", qT_AB), ("d_kAB", kT_AB),
                           ("d_vaug", v_aug)]:
                tmp = sb.tile(list(t_.shape), f32, tag=f"t{nm}", name=f"t{nm}")
                nc.vector.tensor_copy(tmp[:], t_[:])
                nc.sync.dma_start(dbg_out[nm], tmp[:])
            for nm, t_ in [("d_qC", qT_C), ("d_kC2", kC2)]:
                tmp = sb.tile([64, T], f32, tag=f"t{nm}", name=f"t{nm}")
                nc.vector.tensor_copy(tmp[0:64, :], t_[0:64, :])
                nc.sync.dma_start(dbg_out[nm], tmp[0:64, :])

        # ---------------- phase 3: attention + projection ------------------
        with tc.tile_pool(name="scp", bufs=2, space="PSUM") as scp, \
             tc.tile_pool(name="attp", bufs=2, space="PSUM") as attp, \
             tc.tile_pool(name="trp", bufs=1, space="PSUM") as trp, \
             tc.tile_pool(name="ebp", bufs=8) as ebp, \
             tc.tile_pool(name="ebfp", bufs=4) as ebfp, \
             tc.tile_pool(name="anp", bufs=2) as anp, \
             tc.tile_pool(name="rcpp", bufs=6) as rcpp, \
             tc.tile_pool(name="ysp", bufs=4) as ysp:

            # 2 banks of manually sub-allocated scratch: transposes (bf16
            # view of f32 cols 0:256, h01/h2 sequential) + projection psum
            # thirds (pyA f32 cols 256:512, pyB 512:768). Tile dep-tracking
            # is range-precise, so disjoint sub-ranges don't serialize.
            trx = trp.tile([128, 1024], f32, tag="trx", name="trx")
            trx_bf = trx[:, 0:256].bitcast(bf16)  # [128, 512] bf16
            exp_ctr = [0]

            def emit_exp(specs):
                """specs: list of (sc_ap, eb_ap, ebf_cols) with matching
                shapes; one score tile, routed to Act or DVE. Returns True
                if the Act path was used."""
                use_act = EXP_PAT[exp_ctr[0] % len(EXP_PAT)]
                exp_ctr[0] += 1
                if use_act:
                    for sc_ap, eb_ap, _ in specs:
                        nc.scalar.activation(eb_ap, sc_ap, AF.Exp,
                                             scale=SCALE)
                else:
                    ebf = ebfp.tile([128, 1024], f32, tag="ebf", name="ebf")
                    for sc_ap, eb_ap, cols in specs:
                        ebf_ap = ebf[:, cols]
                        if len(sc_ap.shape) == 3:
                            ebf_ap = ebf_ap.rearrange(
                                "p (h c) -> p h c", h=sc_ap.shape[1])
                        nc.vector.tensor_scalar(ebf_ap, sc_ap,
                                                A_SCH, B_SCH,
                                                Alu.mult, Alu.add)
                        nc.vector.tensor_copy(eb_ap.bitcast(i16), ebf_ap)
                return use_act

            proj_pending = []
            py_ctr = [0]

            def emit_proj():
                if not proj_pending:
                    return
                m = proj_pending.pop(0)
                msl = bass.ts(m, 128)
                y_sb = ysp.tile([128, C], f32, tag="ysb", name="ysb")
                for third in range(3):
                    off = 256 + 256 * (py_ctr[0] % 2)
                    py_ctr[0] += 1
                    py = trx[:, off:off + 256]
                    csl = slice(third * 256, (third + 1) * 256)
                    nc.tensor.matmul(py, aoT01[:, msl], wp01[:, csl],
                                     start=True, stop=False,
                                     skip_group_check=True)
                    nc.tensor.matmul(py, aoT2[0:64, msl], wp2[0:64, csl],
                                     start=False, stop=True,
                                     skip_group_check=True)
                    nc.vector.tensor_copy(y_sb[:, csl], py)
                nc.sync.dma_start(y_d[m * 128:(m + 1) * 128, :], y_sb[:])

            # ---- flat software-pipelined tile stream across all j,
            # heads 0,1 (per k-tile) and head 2 (per k-tile pair).
            # Emission order per step: consume(i) -> posts(i) ->
            # produce(i+2), keeping 2 score tiles in flight so att@v never
            # waits on exp and PSUM slot reuse (WAR) never blocks.
            stream = []
            for j in range(NT):
                nk = 4 * j + 4
                for ki in range(nk):
                    stream.append(("01", j, ki))
                for kp in range(nk // 2):
                    stream.append(("2", j, kp))
            NTILES = len(stream)

            st = {}  # per-j live tiles: att01, att2, eb tiles

            def produce(i):
                kind, j, k = stream[i]
                nk = 4 * j + 4
                if kind == "01":
                    ki = k
                    if ki == 0:
                        att01 = [attp.tile([128, 512], f32, tag="att",
                                           name=f"att{h}_{j}")
                                 for h in range(2)]
                        for h in range(2):
                            nc.vector.memset(att01[h][:, 0:260], 0.0)
                        st[("att01", j)] = att01
                    r = ki - 4 * j
                    ksl = bass.ts(ki, 128)
                    trim = 128 * r if r >= 0 else 0
                    w = 512 - trim
                    sc = scp.tile([128, 1024], f32, tag="sc", name="sc")
                    for h in range(2):
                        hp = slice(64 * h, 64 * h + 64)
                        nc.tensor.matmul(
                            sc[:, 512 * h + trim:512 * h + 512],
                            kT_AB[hp, ksl],
                            qT_AB[hp, j * 512 + trim:(j + 1) * 512],
                            start=True, stop=True)
                    eb = ebp.tile([128, 1024], bf16, tag="eb", name="eb")
                    sc_ap = sc[:].rearrange(
                        "p (h c) -> p h c", h=2)[:, :, trim:512]
                    eb_ap = eb[:].rearrange(
                        "p (h c) -> p h c", h=2)[:, :, trim:512]
                    used_act = emit_exp([(sc_ap, eb_ap, slice(0, 2 * w))])
                    if 0 <= r < 4:
                        for h in range(2):
                            strip = slice(512 * h + trim,
                                          512 * h + trim + 128)
                            nc.gpsimd.tensor_mul(eb[:, strip], eb[:, strip],
                                                 cmask[:])
                    st[("eb", i)] = eb
                    st[("lag", i)] = 4 if used_act else 6
                else:
                    kp = k
                    if kp == 0:
                        att2 = attp.tile([128, 512], f32, tag="att",
                                         name=f"att2_{j}")
                        nc.vector.memset(att2[:, 0:260], 0.0)
                        st[("att2", j)] = att2
                    sc = scp.tile([128, 1024], f32, tag="sc", name="sc2")
                    eb = ebp.tile([128, 1024], bf16, tag="eb", name="eb2")
                    kis = (2 * kp, 2 * kp + 1)
                    specs = []
                    for half, ki in enumerate(kis):
                        r = ki - 4 * j
                        ksl = bass.ts(ki, 128)
                        trim = 128 * r if r >= 0 else 0
                        nc.tensor.matmul(
                            sc[:, 512 * half + trim:512 * half + 512],
                            kC2[0:64, ksl],
                            qT_C[0:64, j * 512 + trim:(j + 1) * 512],
                            start=True, stop=True)
                        specs.append(
                            (sc[:, 512 * half + trim:512 * half + 512],
                             eb[:, 512 * half + trim:512 * half + 512],
                             slice(512 * half + trim, 512 * half + 512)))
                    if specs[0][2] == slice(0, 512) and \
                       specs[1][2] == slice(512, 1024):
                        specs = [(sc[:], eb[:], slice(0, 1024))]
                    used_act = emit_exp(specs)
                    for half, ki in enumerate(kis):
                        r = ki - 4 * j
                        if 0 <= r < 4:
                            trim = 128 * r
                            strip = slice(512 * half + trim,
                                          512 * half + trim + 128)
                            nc.gpsimd.tensor_mul(eb[:, strip], eb[:, strip],
                                                 cmask[:])
                    st[("eb", i)] = eb
                    st[("lag", i)] = 4 if used_act else 6

            def consume(i):
                kind, j, k = stream[i]
                eb = st.pop(("eb", i))
                st.pop(("lag", i), None)
                if kind == "01":
                    ki = k
                    r = ki - 4 * j
                    att01 = st[("att01", j)]
                    for h in range(2):
                        for c4 in range(4):
                            if r >= 0 and c4 < r:
                                continue
                            nc.tensor.matmul(
                                att01[h][:, c4 * 65:c4 * 65 + 65],
                                eb[:, 512 * h + 128 * c4:
                                   512 * h + 128 * c4 + 128],
                                v_aug[:, ki * 195 + 65 * h:
                                      ki * 195 + 65 * h + 65],
                                start=False, stop=(ki == 4 * j + c4),
                                skip_group_check=True)
                else:
                    kp = k
                    att2 = st[("att2", j)]
                    for half, ki in enumerate((2 * kp, 2 * kp + 1)):
                        r = ki - 4 * j
                        for c4 in range(4):
                            if r >= 0 and c4 < r:
                                continue
                            nc.tensor.matmul(
                                att2[:, c4 * 65:c4 * 65 + 65],
                                eb[:, 512 * half + 128 * c4:
                                   512 * half + 128 * c4 + 128],
                                v_aug[:, ki * 195 + 130:ki * 195 + 195],
                                start=False, stop=(ki == 4 * j + c4),
                                skip_group_check=True)

            def posts(i):
                kind, j, k = stream[i]
                nk = 4 * j + 4
                jsl = bass.ts(j, 512)
                if kind == "01" and k == nk - 1:
                    if dbg and j == 0:
                        att01 = st[("att01", j)]
                        att_t = ebp.tile([128, 512], f32, tag="attt",
                                         name="attt")
                        nc.vector.tensor_copy(att_t[:], att01[0][:])
                        nc.sync.dma_start(dbg_out["d_att"], att_t[:])
                    # normalize + transpose heads 0,1
                    att01 = st.pop(("att01", j))
                    an01 = anp.tile([128, 512], bf16, tag="an01",
                                    name="an01")
                    for h in range(2):
                        rcp = rcpp.tile([128, 4], f32, tag="rcp",
                                        name="rcp")
                        at = att01[h][:]
                        den = bass.AP(at.tensor, at.offset + 64,
                                      [at.ap[0], [65, 4]])
                        nc.vector.reciprocal_approx_fast(out=rcp[:],
                                                         in_=den)
                        for c4 in range(4):
                            nc.vector.tensor_scalar_mul(
                                an01[:, c4 * 128 + 64 * h:
                                     c4 * 128 + 64 * h + 64],
                                att01[h][:, c4 * 65:c4 * 65 + 64],
                                rcp[:, c4:c4 + 1])
                    tr01 = trx_bf[:, 0:512]
                    for c4 in range(4):
                        csl = bass.ts(c4, 128)
                        nc.tensor.transpose(tr01[:, csl], an01[:, csl],
                                            ident[:])
                    nc.vector.tensor_copy(aoT01[:, jsl], tr01)
                elif kind == "2" and k == nk // 2 - 1:
                    # normalize + transpose head 2
                    att2 = st.pop(("att2", j))
                    an2 = anp.tile([128, 256], bf16, tag="an2", name="an2")
                    rcp2 = rcpp.tile([128, 4], f32, tag="rcp", name="rcp2")
                    at2 = att2[:]
                    den2 = bass.AP(at2.tensor, at2.offset + 64,
                                   [at2.ap[0], [65, 4]])
                    nc.vector.reciprocal_approx_fast(out=rcp2[:], in_=den2)
                    for c4 in range(4):
                        nc.vector.tensor_scalar_mul(
                            an2[:, c4 * 64:c4 * 64 + 64],
                            att2[:, c4 * 65:c4 * 65 + 64],
                            rcp2[:, c4:c4 + 1])
                    tr2 = trx_bf[0:64, 0:512]
                    for c4 in range(4):
                        nc.tensor.transpose(tr2[:, bass.ts(c4, 128)],
                                            an2[:, bass.ts(c4, 64)],
                                            ident[:])
                    nc.vector.tensor_copy(aoT2[0:64, jsl], tr2)
                    proj_pending.extend(range(4 * j, 4 * j + 4))

            produce(0)
            if dbg:
                ebt = ebp.tile([128, 1024], f32, tag="ebt", name="ebt")
                nc.vector.tensor_copy(ebt[:], st[("eb", 0)][:])
                nc.sync.dma_start(dbg_out["d_eb"], ebt[:])
            produce(1)
            next_consume = 0

            def drain_due(step, force=False):
                # consume in order every tile whose lag has expired; a tile
                # with unexpired lag blocks later ones (in-order att stop
                # flags within each section stay safe because lag only
                # reorders across exp paths, not the att accumulate order
                # requirement, which is none: start=False adds onto memory)
                nonlocal next_consume
                while next_consume <= step - st.get(("lag", next_consume), 4) \
                        or (force and next_consume < NTILES):
                    if next_consume >= NTILES or ("eb", next_consume) not in st:
                        break
                    i = next_consume
                    next_consume += 1
                    consume(i)
                    posts(i)
                    if i % 2 == 0:
                        emit_proj()

            for step in range(2, NTILES):
                produce(step)
                drain_due(step)
            drain_due(NTILES + 8, force=True)

            while proj_pending:
                emit_proj()
            if dbg:
                t1 = sb.tile([128, T], f32, tag="tao01")
                nc.vector.tensor_copy(t1[:], aoT01[:])
                nc.sync.dma_start(dbg_out["d_ao01"], t1[:])
                t2 = sb.tile([64, T], f32, tag="tao2")
                nc.vector.tensor_copy(t2[0:64, :], aoT2[0:64, :])
                nc.sync.dma_start(dbg_out["d_ao2"], t2[0:64, :])

    nc.compile()
    return nc


_NC_CACHE = {}


def _get_nc(T):
    if T not in _NC_CACHE:
        _NC_CACHE[T] = build_nc(T)
    return _NC_CACHE[T]


def make_core_inputs(x, W_attn, b_attn, W_proj):
    """Host-side prep: per-core input dicts (see module docstring)."""
    B, T, _ = x.shape
    xts = [np.ascontiguousarray(x[b].T).astype(ml_dtypes.bfloat16)
           for b in range(B)]
    # reference splits qkv as (k, q, v)
    Wk, Wq, Wv = W_attn[:, 0:C], W_attn[:, C:2 * C], W_attn[:, 2 * C:3 * C]
    bq_full = b_attn[C:2 * C]
    in_maps = []
    for core in range(N_CORES):
        b = core // (N_CORES // 2)
        h0 = HPC * (core % (N_CORES // 2))
        ccols = slice(h0 * D, (h0 + 2) * D)
        c2 = slice((h0 + 2) * D, (h0 + 3) * D)
        wq = np.concatenate(
            [Wq[:, ccols], Wk[:, ccols], Wq[:, c2], Wk[:, c2],
             Wv[:, h0 * D:(h0 + 3) * D]], axis=1).astype(ml_dtypes.bfloat16)
        bq = np.zeros((128, 2), np.float32)
        bq[:, 0] = bq_full[ccols]
        bq[0:64, 1] = bq_full[c2]
        wp01 = np.ascontiguousarray(
            W_proj[h0 * D:(h0 + 2) * D, :]).astype(ml_dtypes.bfloat16)
        wp2 = np.ascontiguousarray(
            W_proj[(h0 + 2) * D:(h0 + 3) * D, :]).astype(ml_dtypes.bfloat16)
        in_maps.append({"xt": xts[b], "wq": np.ascontiguousarray(wq),
                        "bq": bq, "wp01": wp01, "wp2": wp2})
    return in_maps


def kernel(x, W_attn, b_attn, W_proj, b_proj):
    x = np.asarray(x, dtype=np.float32)
    W_attn = np.asarray(W_attn, dtype=np.float32)
    b_attn = np.asarray(b_attn, dtype=np.float32)
    W_proj = np.asarray(W_proj, dtype=np.float32)
    b_proj = np.asarray(b_proj, dtype=np.float32)
    B, T, _ = x.shape

    nc = _get_nc(T)
    in_maps = make_core_inputs(x, W_attn, b_attn, W_proj)
    res = None
    for attempt in range(3):
        try:
            res = run_bass_kernel_spmd(nc, in_maps, list(range(N_CORES)))
            break
        except Exception:
            if attempt == 2:
                raise
    global LAST_RUN
    LAST_RUN = res

    gpb = N_CORES // B
    # v-bias folded: softmax weights sum to 1 per row
    b_eff = b_proj + b_attn[2 * C:3 * C] @ W_proj
    out = np.empty((B, T, C), np.float32)
    for b in range(B):
        acc = res.results[b * gpb]["y"].astype(np.float32)
        for g in range(1, gpb):
            acc = acc + res.results[b * gpb + g]["y"]
        out[b] = acc + b_eff[None, :]
    return out


# revision 34
# speedup vs baseline: 1.1543x; 1.0002x over previous
"""Causal self-attention (B=2, T=4096, C=768, H=12) on 8 TRN2 NeuronCores.

Sharding: batch x head-group. Core c handles batch b=c//4 and heads
h0..h0+2 where h0 = 3*(c%4). Each core computes the qkv projection for
its 3 heads, full causal attention, and a partial output projection; the
host sums the 4 partials per batch and adds the (augmented) projection
bias.

Numerics / structure:
- All matmul operands are bf16 (f32 PSUM accumulation).
- k-bias is dropped entirely: softmax over k is invariant to the
  per-query constant q . b_k. v-bias is folded into the host-side output
  bias (softmax weights sum to 1, so + b_v @ W_proj).
- q/k live transposed ([d, T]) feeding the scores matmul; v is computed
  directly in natural layout ([T, d]) with an interleaved ones column
  per head providing the softmax denominator through the att@v matmul.
- scores^T tiles [128 k, 512 q] -> exp -> eb (bf16). exp runs on the Act
  engine for ~3/4 of tiles and as a Schraudolph int16-bitcast
  approximation (tensor_scalar mult+add, then f32->int16 convert copy)
  on DVE for the rest, keeping both engines busy.
- att@v is "flipped": out [128 q, 65] accumulated over k-tiles (65
  cycles per matmul instead of 512), using eb chunks as the stationary
  operand. PSUM accumulation uses pre-zeroed banks + start=False
  matmuls: a start=True wipes the whole bank's in-flight accumulation,
  so interleaved per-chunk groups in one bank need memory-accumulate.
  Diagonal tiles skip fully-masked chunks/columns.
- Normalization uses the denominator column as a per-partition scalar
  (reciprocal + tensor_scalar_mul), then PE transposes the normalized
  [q, d] chunks to d-major for the output projection.
- Phase 3 is software-pipelined: scores/exp for tile ki are emitted one
  step ahead of the att@v consuming ki-1, so att@v sem-waits (4-deep PE
  wait station) never block the next tile's score matmuls.
"""

import sys

for _p in ("/opt/trn_rl_repo",):
    if _p not in sys.path:
        sys.path.insert(0, _p)

import math
import os
from contextlib import ExitStack

import numpy as np
import ml_dtypes

import concourse.bass as bass
import concourse.mybir as mybir
import concourse.tile as tile
from concourse import bacc
from concourse.bass_utils import run_bass_kernel_spmd
from concourse.masks import make_identity

f32 = mybir.dt.float32
bf16 = mybir.dt.bfloat16
i16 = mybir.dt.int16
AF = mybir.ActivationFunctionType
Alu = mybir.AluOpType

C = 768
D = 64
HPC = 3  # heads per core
N_CORES = 8
CK = C // 128  # contraction chunks

SCALE = 1.0 / math.sqrt(D)  # 0.125
# Schraudolph: exp(s*SCALE) ~ bitcast_bf16(int16(s*A_SCH + B_SCH))
A_SCH = SCALE * 128.0 / math.log(2.0)
B_SCH = 128.0 * (127.0 - 0.043)

# exp path pattern over score tiles: True -> Act exp, False -> DVE
# Schraudolph (2 DVE ops).
EXP_PAT = [True, True, True, True, False]


def build_nc(T):
    NT = T // 512  # q tiles
    KT = T // 128  # k tiles
    T2 = T // 2

    nc = bacc.Bacc("TRN2", target_bir_lowering=False, debug=False,
                   num_devices=N_CORES)
    xt_d = nc.dram_tensor("xt", [C, T], bf16, kind="ExternalInput").ap()
    wq_d = nc.dram_tensor("wq", [C, 576], bf16, kind="ExternalInput").ap()
    bq_d = nc.dram_tensor("bq", [128, 2], f32, kind="ExternalInput").ap()
    wp01_d = nc.dram_tensor("wp01", [128, C], bf16, kind="ExternalInput").ap()
    wp2_d = nc.dram_tensor("wp2", [64, C], bf16, kind="ExternalInput").ap()
    y_d = nc.dram_tensor("y", [T, C], f32, kind="ExternalOutput").ap()
    dbg = os.environ.get("KDBG") == "1"
    dbg_out = {}
    if dbg:
        for nm, shp in [("d_qAB", [128, T]), ("d_kAB", [128, T]),
                        ("d_qC", [64, T]), ("d_kC2", [64, T]),
                        ("d_vaug", [128, KT * 195]),
                        ("d_ao01", [128, T]), ("d_ao2", [64, T]),
                        ("d_eb", [128, 1024]), ("d_att", [128, 512])]:
            dbg_out[nm] = nc.dram_tensor(nm, shp, f32, kind="ExternalOutput").ap()

    with tile.TileContext(nc) as tc, ExitStack() as ctx:
        sb = ctx.enter_context(tc.tile_pool(name="sb", bufs=1))

        # persistent tensors
        bq_sb = sb.tile([128, 2], f32, tag="bq")
        qT_AB = sb.tile([128, T], bf16, tag="qAB")
        kT_AB = sb.tile([128, T], bf16, tag="kAB")
        qT_C = sb.tile([64, T], bf16, tag="qC")
        kC2 = sb.tile([64, T], bf16, tag="kC2")
        v_aug = sb.tile([128, KT * 195], bf16, tag="vaug")
        aoT01 = sb.tile([128, T], bf16, tag="aoT01")
        aoT2 = sb.tile([64, T], bf16, tag="aoT2")
        ident = sb.tile([128, 128], bf16, tag="ident")
        cmask = sb.tile([128, 128], bf16, tag="cmask")

        nc.sync.dma_start(bq_sb[:], bq_d)
        make_identity(nc, ident[:])
        # causal triangle for the in-diagonal 128-col strip: keep col >= part
        nc.gpsimd.memset(cmask[:], 1.0)
        nc.gpsimd.affine_select(
            cmask[:], cmask[:], pattern=[[1, 128]],
            compare_op=Alu.is_ge, fill=0.0, base=0, channel_multiplier=-1)
        # ones columns of v_aug (denominator rows through att@v)
        ones_cols = v_aug[:].rearrange("p (k h c) -> p k h c", h=3, c=65)[:, :, :, 64:65]
        nc.gpsimd.memset(ones_cols, 1.0)

        wpp = ctx.enter_context(tc.tile_pool(name="wpp", bufs=1))
        wp01 = wpp.tile([128, C], bf16, tag="wp01")
        wp2 = wpp.tile([64, C], bf16, tag="wp2")

        es_p1 = ExitStack()
        xtp = es_p1.enter_context(tc.tile_pool(name="xtp", bufs=1))
        wqp = es_p1.enter_context(tc.tile_pool(name="wqp", bufs=1))
        k2p = es_p1.enter_context(tc.tile_pool(name="k2p", bufs=1))
        xt_sb = [xtp.tile([128, T], bf16, tag=f"xt{c}", name=f"xt{c}")
                 for c in range(CK)]
        wq_sb = [wqp.tile([128, 576], bf16, tag=f"wq{c}", name=f"wq{c}")
                 for c in range(CK)]
        k2s = k2p.tile([128, T], bf16, tag="k2s")
        for c in range(CK):
            nc.sync.dma_start(wq_sb[c][:], wq_d[c * 128:(c + 1) * 128, :])
        for c in range(CK):
            nc.sync.dma_start(xt_sb[c][:, 0:512],
                              xt_d[c * 128:(c + 1) * 128, 0:512])
        for h0_, h1_ in ((512, T2), (T2, T)):
            if h1_ <= h0_:
                continue
            for c in range(CK):
                nc.sync.dma_start(xt_sb[c][:, h0_:h1_],
                                  xt_d[c * 128:(c + 1) * 128, h0_:h1_])
        nc.sync.dma_start(wp01[:], wp01_d)
        nc.sync.dma_start(wp2[:], wp2_d)

        # ---------------- phase 1: qkv projection --------------------------
        with tc.tile_pool(name="qkp", bufs=2, space="PSUM") as qkp, \
             tc.tile_pool(name="vps", bufs=2, space="PSUM") as vps:
            for j in range(NT):
                jsl = bass.ts(j, 512)
                q01 = qkp.tile([128, 512], f32, tag="q01", name="q01")
                k01 = qkp.tile([128, 512], f32, tag="k01", name="k01")
                qk2 = qkp.tile([128, 512], f32, tag="qk2", name="qk2")
                for c in range(CK):
                    st, sp = c == 0, c == CK - 1
                    nc.tensor.matmul(q01[:], wq_sb[c][:, 0:128],
                                     xt_sb[c][:, jsl], start=st, stop=sp)
                    nc.tensor.matmul(k01[:], wq_sb[c][:, 128:256],
                                     xt_sb[c][:, jsl], start=st, stop=sp)
                    nc.tensor.matmul(qk2[:], wq_sb[c][:, 256:384],
                                     xt_sb[c][:, jsl], start=st, stop=sp)
                nc.vector.tensor_scalar_add(qT_AB[:, jsl], q01[:],
                                            bq_sb[:, 0:1])
                nc.vector.tensor_copy(kT_AB[:, jsl], k01[:])
                nc.vector.tensor_scalar_add(qT_C[0:64, jsl], qk2[0:64, :],
                                            bq_sb[0:64, 1:2])
                nc.vector.tensor_copy(k2s[64:128, jsl], qk2[64:128, :])
                for mi in range(4):
                    m = 4 * j + mi
                    msl = bass.ts(m, 128)
                    vp = vps.tile([128, 192], f32, tag="vp", name="vp")
                    for c in range(CK):
                        nc.tensor.matmul(vp[:], xt_sb[c][:, msl],
                                         wq_sb[c][:, 384:576],
                                         start=(c == 0), stop=(c == CK - 1))
                    vdst = v_aug[:, m * 195:(m + 1) * 195].rearrange(
                        "p (h c) -> p h c", c=65)[:, :, 0:64]
                    nc.vector.tensor_copy(
                        vdst, vp[:].rearrange("p (h c) -> p h c", c=64))
        # k2: partition shift 64:128 -> 0:64 via sbuf->sbuf DMA
        nc.sync.dma_start(kC2[0:64, :], k2s[64:128, :])
        es_p1.close()
        if dbg:
            for nm, t_ in [("d_qAB", qT_AB), ("d_kAB", kT_AB),
                           ("d_vaug", v_aug)]:
                tmp = sb.tile(list(t_.shape), f32, tag=f"t{nm}", name=f"t{nm}")
                nc.vector.tensor_copy(tmp[:], t_[:])
                nc.sync.dma_start(dbg_out[nm], tmp[:])
            for nm, t_ in [("d_qC", qT_C), ("d_kC2", kC2)]:
                tmp = sb.tile([64, T], f32, tag=f"t{nm}", name=f"t{nm}")
                nc.vector.tensor_copy(tmp[0:64, :], t_[0:64, :])
                nc.sync.dma_start(dbg_out[nm], tmp[0:64, :])

        # ---------------- phase 3: attention + projection ------------------
        with tc.tile_pool(name="scp", bufs=2, space="PSUM") as scp, \
             tc.tile_pool(name="attp", bufs=2, space="PSUM") as attp, \
             tc.tile_pool(name="trp", bufs=1, space="PSUM") as trp, \
             tc.tile_pool(name="ebp", bufs=7) as ebp, \
             tc.tile_pool(name="ebfp", bufs=3) as ebfp, \
             tc.tile_pool(name="anp", bufs=2) as anp, \
             tc.tile_pool(name="rcpp", bufs=6) as rcpp, \
             tc.tile_pool(name="ysp", bufs=4) as ysp:

            # 2 banks of manually sub-allocated scratch: transposes (bf16
            # view of f32 cols 0:256, h01/h2 sequential) + projection psum
            # thirds (pyA f32 cols 256:512, pyB 512:768). Tile dep-tracking
            # is range-precise, so disjoint sub-ranges don't serialize.
            trx = trp.tile([128, 1024], f32, tag="trx", name="trx")
            trx_bf = trx[:, 0:256].bitcast(bf16)  # [128, 512] bf16
            exp_ctr = [0]

            def emit_exp(specs):
                """specs: list of (sc_ap, eb_ap, ebf_cols) with matching
                shapes; one score tile, routed to Act or DVE. Returns True
                if the Act path was used."""
                use_act = EXP_PAT[exp_ctr[0] % len(EXP_PAT)]
                exp_ctr[0] += 1
                if use_act:
                    for sc_ap, eb_ap, _ in specs:
                        nc.scalar.activation(eb_ap, sc_ap, AF.Exp,
                                             scale=SCALE)
                else:
                    ebf = ebfp.tile([128, 1024], f32, tag="ebf", name="ebf")
                    for sc_ap, eb_ap, cols in specs:
                        ebf_ap = ebf[:, cols]
                        if len(sc_ap.shape) == 3:
                            ebf_ap = ebf_ap.rearrange(
                                "p (h c) -> p h c", h=sc_ap.shape[1])
                        nc.vector.tensor_scalar(ebf_ap, sc_ap,
                                                A_SCH, B_SCH,
                                                Alu.mult, Alu.add)
                        nc.vector.tensor_copy(eb_ap.bitcast(i16), ebf_ap)
                return use_act

            proj_pending = []
            py_ctr = [0]

            def emit_proj():
                if not proj_pending:
                    return
                m = proj_pending.pop(0)
                msl = bass.ts(m, 128)
                y_sb = ysp.tile([128, C], f32, tag="ysb", name="ysb")
                for third in range(3):
                    off = 256 + 256 * (py_ctr[0] % 2)
                    py_ctr[0] += 1
                    py = trx[:, off:off + 256]
                    csl = slice(third * 256, (third + 1) * 256)
                    nc.tensor.matmul(py, aoT01[:, msl], wp01[:, csl],
                                     start=True, stop=False,
                                     skip_group_check=True)
                    nc.tensor.matmul(py, aoT2[0:64, msl], wp2[0:64, csl],
                                     start=False, stop=True,
                                     skip_group_check=True)
                    nc.vector.tensor_copy(y_sb[:, csl], py)
                nc.sync.dma_start(y_d[m * 128:(m + 1) * 128, :], y_sb[:])

            # ---- flat software-pipelined tile stream across all j,
            # heads 0,1 (per k-tile) and head 2 (per k-tile pair).
            # Emission order per step: consume(i) -> posts(i) ->
            # produce(i+2), keeping 2 score tiles in flight so att@v never
            # waits on exp and PSUM slot reuse (WAR) never blocks.
            stream = []
            for j in range(NT):
                nk = 4 * j + 4
                for ki in range(nk):
                    stream.append(("01", j, ki))
                for kp in range(nk // 2):
                    stream.append(("2", j, kp))
            NTILES = len(stream)

            st = {}  # per-j live tiles: att01, att2, eb tiles

            def produce(i):
                kind, j, k = stream[i]
                nk = 4 * j + 4
                if kind == "01":
                    ki = k
                    if ki == 0:
                        att01 = [attp.tile([128, 512], f32, tag="att",
                                           name=f"att{h}_{j}")
                                 for h in range(2)]
                        for h in range(2):
                            nc.vector.memset(att01[h][:, 0:260], 0.0)
                        st[("att01", j)] = att01
                    r = ki - 4 * j
                    ksl = bass.ts(ki, 128)
                    trim = 128 * r if r >= 0 else 0
                    w = 512 - trim
                    sc = scp.tile([128, 1024], f32, tag="sc", name="sc")
                    for h in range(2):
                        hp = slice(64 * h, 64 * h + 64)
                        nc.tensor.matmul(
                            sc[:, 512 * h + trim:512 * h + 512],
                            kT_AB[hp, ksl],
                            qT_AB[hp, j * 512 + trim:(j + 1) * 512],
                            start=True, stop=True)
                    eb = ebp.tile([128, 1024], bf16, tag="eb", name="eb")
                    sc_ap = sc[:].rearrange(
                        "p (h c) -> p h c", h=2)[:, :, trim:512]
                    eb_ap = eb[:].rearrange(
                        "p (h c) -> p h c", h=2)[:, :, trim:512]
                    used_act = emit_exp([(sc_ap, eb_ap, slice(0, 2 * w))])
                    if 0 <= r < 4:
                        for h in range(2):
                            strip = slice(512 * h + trim,
                                          512 * h + trim + 128)
                            nc.gpsimd.tensor_mul(eb[:, strip], eb[:, strip],
                                                 cmask[:])
                    st[("eb", i)] = eb
                    st[("lag", i)] = 4 if used_act else 5
                else:
                    kp = k
                    if kp == 0:
                        att2 = attp.tile([128, 512], f32, tag="att",
                                         name=f"att2_{j}")
                        nc.vector.memset(att2[:, 0:260], 0.0)
                        st[("att2", j)] = att2
                    sc = scp.tile([128, 1024], f32, tag="sc", name="sc2")
                    eb = ebp.tile([128, 1024], bf16, tag="eb", name="eb2")
                    kis = (2 * kp, 2 * kp + 1)
                    specs = []
                    for half, ki in enumerate(kis):
                        r = ki - 4 * j
                        ksl = bass.ts(ki, 128)
                        trim = 128 * r if r >= 0 else 0
                        nc.tensor.matmul(
                            sc[:, 512 * half + trim:512 * half + 512],
                            kC2[0:64, ksl],
                            qT_C[0:64, j * 512 + trim:(j + 1) * 512],
                            start=True, stop=True)
                        specs.append(
                            (sc[:, 512 * half + trim:512 * half + 512],
                             eb[:, 512 * half + trim:512 * half + 512],
                             slice(512 * half + trim, 512 * half + 512)))
                    if specs[0][2] == slice(0, 512) and \
                       specs[1][2] == slice(512, 1024):
                        specs = [(sc[:], eb[:], slice(0, 1024))]
                    used_act = emit_exp(specs)
                    for half, ki in enumerate(kis):
                        r = ki - 4 * j
                        if 0 <= r < 4:
                            trim = 128 * r
                            strip = slice(512 * half + trim,
                                          512 * half + trim + 128)
                            nc.gpsimd.tensor_mul(eb[:, strip], eb[:, strip],
                                                 cmask[:])
                    st[("eb", i)] = eb
                    st[("lag", i)] = 4 if used_act else 5

            def consume(i):
                kind, j, k = stream[i]
                eb = st.pop(("eb", i))
                st.pop(("lag", i), None)
                if kind == "01":
                    ki = k
                    r = ki - 4 * j
                    att01 = st[("att01", j)]
                    for h in range(2):
                        for c4 in range(4):
                            if r >= 0 and c4 < r:
                                continue
                            nc.tensor.matmul(
                                att01[h][:, c4 * 65:c4 * 65 + 65],
                                eb[:, 512 * h + 128 * c4:
                                   512 * h + 128 * c4 + 128],
                                v_aug[:, ki * 195 + 65 * h:
                                      ki * 195 + 65 * h + 65],
                                start=False, stop=(ki == 4 * j + c4),
                                skip_group_check=True)
                else:
                    kp = k
                    att2 = st[("att2", j)]
                    for half, ki in enumerate((2 * kp, 2 * kp + 1)):
                        r = ki - 4 * j
                        for c4 in range(4):
                            if r >= 0 and c4 < r:
                                continue
                            nc.tensor.matmul(
                                att2[:, c4 * 65:c4 * 65 + 65],
                                eb[:, 512 * half + 128 * c4:
                                   512 * half + 128 * c4 + 128],
                                v_aug[:, ki * 195 + 130:ki * 195 + 195],
                                start=False, stop=(ki == 4 * j + c4),
                                skip_group_check=True)

            def posts(i):
                kind, j, k = stream[i]
                nk = 4 * j + 4
                jsl = bass.ts(j, 512)
                if kind == "01" and k == nk - 1:
                    if dbg and j == 0:
                        att01 = st[("att01", j)]
                        att_t = ebp.tile([128, 512], f32, tag="attt",
                                         name="attt")
                        nc.vector.tensor_copy(att_t[:], att01[0][:])
                        nc.sync.dma_start(dbg_out["d_att"], att_t[:])
                    # normalize + transpose heads 0,1
                    att01 = st.pop(("att01", j))
                    an01 = anp.tile([128, 512], bf16, tag="an01",
                                    name="an01")
                    for h in range(2):
                        rcp = rcpp.tile([128, 4], f32, tag="rcp",
                                        name="rcp")
                        at = att01[h][:]
                        den = bass.AP(at.tensor, at.offset + 64,
                                      [at.ap[0], [65, 4]])
                        nc.vector.reciprocal_approx_fast(out=rcp[:],
                                                         in_=den)
                        for c4 in range(4):
                            nc.vector.tensor_scalar_mul(
                                an01[:, c4 * 128 + 64 * h:
                                     c4 * 128 + 64 * h + 64],
                                att01[h][:, c4 * 65:c4 * 65 + 64],
                                rcp[:, c4:c4 + 1])
                    tr01 = trx_bf[:, 0:512]
                    for c4 in range(4):
                        csl = bass.ts(c4, 128)
                        nc.tensor.transpose(tr01[:, csl], an01[:, csl],
                                            ident[:])
                    nc.vector.tensor_copy(aoT01[:, jsl], tr01)
                elif kind == "2" and k == nk // 2 - 1:
                    # normalize + transpose head 2
                    att2 = st.pop(("att2", j))
                    an2 = anp.tile([128, 256], bf16, tag="an2", name="an2")
                    rcp2 = rcpp.tile([128, 4], f32, tag="rcp", name="rcp2")
                    at2 = att2[:]
                    den2 = bass.AP(at2.tensor, at2.offset + 64,
                                   [at2.ap[0], [65, 4]])
                    nc.vector.reciprocal_approx_fast(out=rcp2[:], in_=den2)
                    for c4 in range(4):
                        nc.vector.tensor_scalar_mul(
                            an2[:, c4 * 64:c4 * 64 + 64],
                            att2[:, c4 * 65:c4 * 65 + 64],
                            rcp2[:, c4:c4 + 1])
                    tr2 = trx_bf[0:64, 0:512]
                    for c4 in range(4):
                        nc.tensor.transpose(tr2[:, bass.ts(c4, 128)],
                                            an2[:, bass.ts(c4, 64)],
                                            ident[:])
                    nc.vector.tensor_copy(aoT2[0:64, jsl], tr2)
                    proj_pending.extend(range(4 * j, 4 * j + 4))

            produce(0)
            if dbg:
                ebt = ebp.tile([128, 1024], f32, tag="ebt", name="ebt")
                nc.vector.tensor_copy(ebt[:], st[("eb", 0)][:])
                nc.sync.dma_start(dbg_out["d_eb"], ebt[:])
            produce(1)
            next_consume = 0

            def drain_due(step, force=False):
                # consume in order every tile whose lag has expired; a tile
                # with unexpired lag blocks later ones (in-order att stop
                # flags within each section stay safe because lag only
                # reorders across exp paths, not the att accumulate order
                # requirement, which is none: start=False adds onto memory)
                nonlocal next_consume
                while next_consume <= step - st.get(("lag", next_consume), 4) \
                        or (force and next_consume < NTILES):
                    if next_consume >= NTILES or ("eb", next_consume) not in st:
                        break
                    i = next_consume
                    next_consume += 1
                    consume(i)
                    posts(i)
                    if i % 2 == 0:
                        emit_proj()

            for step in range(2, NTILES):
                produce(step)
                drain_due(step)
            drain_due(NTILES + 8, force=True)

            while proj_pending:
                emit_proj()
            if dbg:
                t1 = sb.tile([128, T], f32, tag="tao01")
                nc.vector.tensor_copy(t1[:], aoT01[:])
                nc.sync.dma_start(dbg_out["d_ao01"], t1[:])
                t2 = sb.tile([64, T], f32, tag="tao2")
                nc.vector.tensor_copy(t2[0:64, :], aoT2[0:64, :])
                nc.sync.dma_start(dbg_out["d_ao2"], t2[0:64, :])

    nc.compile()
    return nc


_NC_CACHE = {}


def _get_nc(T):
    if T not in _NC_CACHE:
        _NC_CACHE[T] = build_nc(T)
    return _NC_CACHE[T]


def make_core_inputs(x, W_attn, b_attn, W_proj):
    """Host-side prep: per-core input dicts (see module docstring)."""
    B, T, _ = x.shape
    xts = [np.ascontiguousarray(x[b].T).astype(ml_dtypes.bfloat16)
           for b in range(B)]
    # reference splits qkv as (k, q, v)
    Wk, Wq, Wv = W_attn[:, 0:C], W_attn[:, C:2 * C], W_attn[:, 2 * C:3 * C]
    bq_full = b_attn[C:2 * C]
    in_maps = []
    for core in range(N_CORES):
        b = core // (N_CORES // 2)
        h0 = HPC * (core % (N_CORES // 2))
        ccols = slice(h0 * D, (h0 + 2) * D)
        c2 = slice((h0 + 2) * D, (h0 + 3) * D)
        wq = np.concatenate(
            [Wq[:, ccols], Wk[:, ccols], Wq[:, c2], Wk[:, c2],
             Wv[:, h0 * D:(h0 + 3) * D]], axis=1).astype(ml_dtypes.bfloat16)
        bq = np.zeros((128, 2), np.float32)
        bq[:, 0] = bq_full[ccols]
        bq[0:64, 1] = bq_full[c2]
        wp01 = np.ascontiguousarray(
            W_proj[h0 * D:(h0 + 2) * D, :]).astype(ml_dtypes.bfloat16)
        wp2 = np.ascontiguousarray(
            W_proj[(h0 + 2) * D:(h0 + 3) * D, :]).astype(ml_dtypes.bfloat16)
        in_maps.append({"xt": xts[b], "wq": np.ascontiguousarray(wq),
                        "bq": bq, "wp01": wp01, "wp2": wp2})
    return in_maps


def kernel(x, W_attn, b_attn, W_proj, b_proj):
    x = np.asarray(x, dtype=np.float32)
    W_attn = np.asarray(W_attn, dtype=np.float32)
    b_attn = np.asarray(b_attn, dtype=np.float32)
    W_proj = np.asarray(W_proj, dtype=np.float32)
    b_proj = np.asarray(b_proj, dtype=np.float32)
    B, T, _ = x.shape

    nc = _get_nc(T)
    in_maps = make_core_inputs(x, W_attn, b_attn, W_proj)
    res = None
    for attempt in range(3):
        try:
            res = run_bass_kernel_spmd(nc, in_maps, list(range(N_CORES)))
            break
        except Exception:
            if attempt == 2:
                raise
    global LAST_RUN
    LAST_RUN = res

    gpb = N_CORES // B
    # v-bias folded: softmax weights sum to 1 per row
    b_eff = b_proj + b_attn[2 * C:3 * C] @ W_proj
    out = np.empty((B, T, C), np.float32)
    for b in range(B):
        acc = res.results[b * gpb]["y"].astype(np.float32)
        for g in range(1, gpb):
            acc = acc + res.results[b * gpb + g]["y"]
        out[b] = acc + b_eff[None, :]
    return out
